# revision 11
# baseline (speedup 1.0000x reference)
"""Complex multi-head attention on 8 Trainium2 cores (Bass/Tile).

Sharding: pure data-parallel over batch (B=8 -> 1 batch per core),
weights replicated. No collectives.

v2 redesign (vs baseline at 465us):
  - Every LDWEIGHTS serves 2 matmuls (the two 512-col halves of each
    1024-wide rhs share the stationary operand) so the fp32 weight-load
    (224ns, no FWL for fp32) hides under 2x213ns of streaming.
  - Scores for both halves land in one 2-bank PSUM pair tile and are
    exponentiated with a single [128,1024] activation (amortizes the
    ~300-cycle ACT instruction overhead).
  - Softmax reciprocal moved off the DVE (was 32 x 3.9us of iterative
    divide) to the scalar engine as exp(-ln(sums)); Ln+Exp share one
    ACT table set.
  - P1/P2 evacuated unnormalized; normalization (3 DVE ops/head)
    happens off the critical path during the next head's projections.
  - Lag-1 software pipeline: st pair of iter i+1 is emitted between
    exp(i) and the sums/AV matmuls of iter i, so the PE never waits on
    the scalar engine; next head's K/Q projection chains are spliced
    into the head boundary to cover the last exp + rec latency.
  - O-projection computed transposed (out = [odim, tok]) with
    wo-stationary h-chains so its LDW is also paired; host undoes the
    transpose.
PSUM: st/proj/V/O pair pool [128,1024]x2 (4 banks) + sums pair
[128,1024]x1 (2) + P accumulators [128,512]x2 (2) = 8 banks exactly.
"""

import sys
import types
import numpy as np

B, S, D, H = 8, 1024, 512, 8
DH = D // H
KC = 8  # k-chunks of 128 over (c,d) = 1024
TC = 8  # token chunks of 128
NCORES = 8

LAST_EXEC_NS = None


# ---------------------------------------------------------------- shims
def _install_axon_profile_shim():
    if "antenv.axon_hooks" in sys.modules:
        return
    try:
        import antenv  # noqa: F401

        mod = types.ModuleType("antenv.axon_hooks")
        state = {"hook": None}
        mod.set_axon_ntff_profile_hook = lambda h: state.__setitem__("hook", h)
        mod.get_axon_ntff_profile_hook = lambda: state["hook"]
        sys.modules["antenv.axon_hooks"] = mod
        from trn_agent_boot.trn_boot import _ntff_profile_via_ctypes

        hook = _ntff_profile_via_ctypes("/opt/axon/libaxon_pjrt.so")
        if hook is not None:
            mod.set_axon_ntff_profile_hook(hook)
    except Exception:
        pass


def _install_tile_drain_patch():
    """This walrus build allows ONE sync wait per instruction; split the
    TileContext exit drain's waits across preceding sync NOPs."""
    import concourse.mybir as mybir
    import concourse.tile as tile
    from concourse.vector_clock import ScopedClock

    if getattr(tile.TileContext, "_drain_patched", False):
        return

    def _patched(self, tick_clock, wait_clock):
        probe = mybir.InstNoOp(name="I-drain-probe")
        probe.engine = mybir.EngineType.SP
        wait_clock.add_sem_waits(probe, ScopedClock({None: tick_clock.global_clock}))
        waits = list(probe.sync_info.on_wait or []) if probe.sync_info else []
        for w in waits:
            nop = self.nc.sync.nop()
            nop.ins.sync_info = mybir.SyncInfo(on_wait=[w], on_update=[])
        self.nc.sync.drain()
        self.nc.all_engine_barrier()
        assert self.sems is not None
        popped = self.nc._tile_sem_poison_stack.pop()
        assert popped is self._sem_poison
        self.nc.clear_and_free_semaphores(list(self.sems.allocated().values()))
        self.nc.all_engine_barrier()

    tile.TileContext._drain_and_barrier = _patched
    tile.TileContext._drain_patched = True


def _split_waits(nc, max_waits=1):
    """Hoist extra sync waits onto preceding same-engine NOPs (walrus here
    rejects >1 sync wait per instruction)."""
    import concourse.mybir as mybir

    def process(blk):
        lst = blk.instructions
        i = 0
        while i < len(lst):
            inst = lst[i]
            if hasattr(inst, "blocks"):
                for b in inst.blocks or []:
                    process(b)
            si = inst.sync_info
            if si is not None and si.on_wait and len(si.on_wait) > max_waits:
                waits = list(si.on_wait)
                keep, extra = waits[-max_waits:], waits[:-max_waits]
                inst.sync_info = mybir.SyncInfo(
                    on_wait=keep, on_update=list(si.on_update or [])
                )
                for j, w in enumerate(extra):
                    nop = mybir.InstNoOp(name=f"{inst.name}-ws{j}")
                    nop.engine = inst.engine
                    nop.sync_info = mybir.SyncInfo(on_wait=[w], on_update=[])
                    lst.insert(i, nop)
                    i += 1
            i += 1

    for f in nc.m.functions:
        for blk in f.blocks:
            process(blk)


# ------------------------------------------------------------ host prep
def _build_wqk(wr, wi, scale):
    """[1024 k=(c,d), 1024 m=(h, c', dh)] for Q/K projections."""
    W = np.empty((2 * D, 2 * D), np.float32)
    for h in range(H):
        o = slice(h * DH, (h + 1) * DH)
        c0 = h * 2 * DH
        W[0:D, c0 : c0 + DH] = wr[o].T * scale
        W[D:, c0 : c0 + DH] = -wi[o].T * scale
        W[0:D, c0 + DH : c0 + 2 * DH] = wi[o].T * scale
        W[D:, c0 + DH : c0 + 2 * DH] = wr[o].T * scale
    return W


def _head_tiles(W):
    """[1024,1024] -> [H, 128, 1024]: per-head column block, k-chunk cols."""
    out = np.empty((H, 128, 1024), np.float32)
    for h in range(H):
        blk = W[:, h * 128 : (h + 1) * 128]  # [1024, 128]
        for kk in range(KC):
            out[h, :, kk * 128 : (kk + 1) * 128] = blk[kk * 128 : (kk + 1) * 128]
    return out


def _kchunk_tiles(W):
    """[1024,1024] -> [KC, 128, 1024]: row chunks."""
    return np.ascontiguousarray(W.reshape(KC, 128, 1024))


def _build_wo(wo_r, wo_i):
    """rows (h, c', dh), cols (o, c) interleaved to match [S, D, 2]."""
    W = np.empty((2 * D, 2 * D), np.float32)
    for h in range(H):
        d = slice(h * DH, (h + 1) * DH)
        r0 = h * 2 * DH
        W[r0 : r0 + DH, 0::2] = wo_r[:, d].T
        W[r0 : r0 + DH, 1::2] = wo_i[:, d].T
        W[r0 + DH : r0 + 2 * DH, 0::2] = -wo_i[:, d].T
        W[r0 + DH : r0 + 2 * DH, 1::2] = wo_r[:, d].T
    return W


def _xt(x):  # [S, D, 2] -> [2D, S] feature-major
    out = np.empty((2 * D, S), np.float32)
    out[0:D] = x[:, :, 0].T
    out[D:] = x[:, :, 1].T
    return out


# ------------------------------------------------------------ bass build
def _build_nc():
    import concourse.bass as bass
    import concourse.mybir as mybir
    import concourse.tile as tile
    from contextlib import ExitStack

    MDT = mybir.dt.float32r
    F32 = mybir.dt.float32
    EXP = mybir.ActivationFunctionType.Exp
    LN = mybir.ActivationFunctionType.Ln

    nc = bass.Bass()
    d_xtq = nc.dram_tensor("xtq", [KC, 128, S], MDT, kind="ExternalInput")
    d_xtk = nc.dram_tensor("xtk", [KC, 128, S], MDT, kind="ExternalInput")
    d_xtv = nc.dram_tensor("xtv", [KC, 128, S], MDT, kind="ExternalInput")
    d_wq = nc.dram_tensor("wq", [H, 128, 1024], MDT, kind="ExternalInput")
    d_wk = nc.dram_tensor("wk", [H, 128, 1024], MDT, kind="ExternalInput")
    d_wv = nc.dram_tensor("wv", [KC, 128, 1024], MDT, kind="ExternalInput")
    d_wo = nc.dram_tensor("wo", [H, 128, 1024], MDT, kind="ExternalInput")
    d_cst = nc.dram_tensor("cst", [128, 320], MDT, kind="ExternalInput")
    # transposed output: [oc, odim, tok]
    d_out = nc.dram_tensor("out", [KC, 128, S], F32, kind="ExternalOutput")

    with tile.TileContext(nc) as tc, ExitStack() as ctx:
        ctx.enter_context(
            nc.allow_low_precision(reason="float32r tiles are bit-identical fp32")
        )
        pA = ctx.enter_context(tc.tile_pool(name="bigA", bufs=8))  # xtv->xtq->wo
        pB = ctx.enter_context(tc.tile_pool(name="bigB", bufs=8))  # wv->xtk->oev
        pV1 = ctx.enter_context(tc.tile_pool(name="v1", bufs=1))
        pOsb = ctx.enter_context(tc.tile_pool(name="osb", bufs=1))
        pQs = ctx.enter_context(tc.tile_pool(name="qs", bufs=2))
        pKs = ctx.enter_context(tc.tile_pool(name="ks", bufs=2))
        pQw = ctx.enter_context(tc.tile_pool(name="qw", bufs=1))
        pKn = ctx.enter_context(tc.tile_pool(name="kn", bufs=1))
        pV2 = ctx.enter_context(tc.tile_pool(name="v2", bufs=1))
        pE = ctx.enter_context(tc.tile_pool(name="e", bufs=2))
        pEs = ctx.enter_context(tc.tile_pool(name="es", bufs=2))
        pPsb = ctx.enter_context(tc.tile_pool(name="psb", bufs=2))
        pRec = ctx.enter_context(tc.tile_pool(name="rec", bufs=3))
        pWqk = ctx.enter_context(tc.tile_pool(name="wqk", bufs=3))
        pC = ctx.enter_context(tc.tile_pool(name="const", bufs=1))

        ps_pair = ctx.enter_context(tc.tile_pool(name="ps_pair", bufs=2, space="PSUM"))
        ps_sums = ctx.enter_context(tc.tile_pool(name="ps_sums", bufs=1, space="PSUM"))
        ps_p = ctx.enter_context(tc.tile_pool(name="ps_p", bufs=1, space="PSUM"))

        cst = pC.tile([128, 320], MDT, tag="cst")
        nc.sync.dma_start(out=cst, in_=d_cst[:, :])
        ones128 = cst[:, 0:128]

        def mm(out, lhsT, rhs, start, stop):
            nc.tensor.matmul(out, lhsT=lhsT, rhs=rhs, start=start, stop=stop)

        # ---- phase V: V projection (all heads) ----
        def dma_split(out, in_, n=4):
            w = out.shape[-1] // n
            for i in range(n):
                nc.sync.dma_start(
                    out=out[:, i * w : (i + 1) * w], in_=in_[:, i * w : (i + 1) * w]
                )

        xtv, wv = [], []
        for kk in range(KC):
            tv = pA.tile([128, S], MDT, tag="bigA", name=f"xtv{kk}")
            tw = pB.tile([128, 1024], MDT, tag="bigB", name=f"wv{kk}")
            if kk < 2:
                dma_split(tv, d_xtv[kk], n=2)
                dma_split(tw, d_wv[kk], n=2)
            else:
                nc.sync.dma_start(out=tv, in_=d_xtv[kk])
                nc.sync.dma_start(out=tw, in_=d_wv[kk])
            xtv.append(tv)
            wv.append(tw)

        v1 = pV1.tile([128, TC * 1024], MDT, tag="v1")  # [p, (t_, h, dh2)]
        for t_ in range(TC):
            pair = ps_pair.tile([128, 1024], F32, tag="pair", name=f"vps{t_}")
            tsl = slice(t_ * 128, (t_ + 1) * 128)
            for kk in range(KC):
                lhsT = xtv[kk][:, tsl]
                mm(pair[:, 0:512], lhsT, wv[kk][:, 0:512], kk == 0, kk == KC - 1)
                mm(pair[:, 512:1024], lhsT, wv[kk][:, 512:1024], kk == 0, kk == KC - 1)
            nc.vector.tensor_copy(v1[:, t_ * 1024 : (t_ + 1) * 1024], pair)

        # ---- load XT_q / XT_k (reuse pA / pB slots) ----
        xtq, xtk = [], []

        osb = pOsb.tile([128, H * 1024], MDT, tag="osb")  # [p, (h, tok)]

        # ---------------- per-head helpers ----------------
        wq_t, wk_t = {}, {}

        def prefetch_w(h):
            if h >= H or h in wq_t:
                return
            tk2 = pWqk.tile([128, 1024], MDT, tag="wqk", name=f"wk{h}")
            dma_split(tk2, d_wk[h], n=2)
            wk_t[h] = tk2
            tq = pWqk.tile([128, 1024], MDT, tag="wqk", name=f"wq{h}")
            dma_split(tq, d_wq[h], n=2)
            wq_t[h] = tq

        def proj_chain(w_tile, xt_tiles, name):
            pair = ps_pair.tile([128, 1024], F32, tag="pair", name=name)
            for kk in range(KC):
                lhsT = w_tile[:, kk * 128 : (kk + 1) * 128]
                mm(pair[:, 0:512], lhsT, xt_tiles[kk][:, 0:512], kk == 0, kk == KC - 1)
                mm(
                    pair[:, 512:1024],
                    lhsT,
                    xt_tiles[kk][:, 512:1024],
                    kk == 0,
                    kk == KC - 1,
                )
            return pair

        def emit_k_chain(h):
            return proj_chain(wk_t.pop(h), xtk, f"kproj{h}")

        def finish_k(h, pair):
            ks = pKs.tile([128, S], MDT, tag="ks", name=f"kstack{h}")
            nc.vector.tensor_copy(ks, pair)
            kn = pKn.tile([128, S], MDT, tag="kn", name=f"kneg{h}")
            nc.vector.tensor_copy(kn[0:64, :], ks[0:64, :])
            nc.vector.tensor_scalar_mul(kn[64:128, :], ks[64:128, :], -1.0)
            return ks, kn

        def emit_q_chain(h):
            return proj_chain(wq_t.pop(h), xtq, f"qproj{h}")

        def finish_q(h, pair):
            qs = pQs.tile([128, S], MDT, tag="qs", name=f"qstack{h}")
            nc.vector.tensor_copy(qs[:, 0:512], pair[:, 0:512])
            nc.vector.tensor_copy(qs[:, 512:1024], pair[:, 512:1024])
            qw = pQw.tile([128, S], MDT, tag="qw", name=f"qswap{h}")
            nc.sync.dma_start(out=qw[0:64, :], in_=qs[64:128, :])
            nc.sync.dma_start(out=qw[64:128, :], in_=qs[0:64, :])
            return qs, qw

        def build_v2(h):
            v2t = pV2.tile([128, 1024], MDT, tag="v2", name=f"v2h{h}")
            v1v = v1.rearrange("p (t h d) -> p t h d", t=TC, h=H, d=128)
            v2v = v2t.rearrange("p (t d) -> p t d", t=TC, d=128)
            nc.vector.tensor_scalar_mul(
                v2v[:, :, 0:64], v1v[:, :, h, 64:128], -1.0
            )
            nc.vector.tensor_copy(v2v[:, :, 64:128], v1v[:, :, h, 0:64])
            return v2t

        # ---------------- the pipelined head loop ----------------
        # pending = (sums, p0, p1, e, vt, first, last, boundary_cb)
        state = {"pending": None, "post": None}

        def flush_post():
            flush()
            sp, boundary = state["post"]
            emit_sums(sp)
            boundary()
            state["post"] = None

        def flush():
            p = state["pending"]
            if p is None:
                return
            p0, p1, e, vt, first, last = p
            mm(p0, vt, e[:, 0:512], first, last)
            mm(p1, vt, e[:, 512:1024], first, last)
            state["pending"] = None

        def emit_sums(sp):
            sums, esum, first, last = sp
            mm(sums[:, 0:512], ones128, esum[:, 0:512], first, last)
            mm(sums[:, 512:1024], ones128, esum[:, 512:1024], first, last)

        def make_boundary(h, comp, sums, pp, recs, psbs):
            def boundary():
                lnt = pRec.tile([128, 1024], MDT, tag="rec", name=f"lnt{h}_{comp}")
                nc.scalar.activation(lnt, sums, func=LN)
                rec = pRec.tile([128, 1024], MDT, tag="rec", name=f"rec{h}_{comp}")
                nc.scalar.activation(rec, lnt, func=EXP, scale=-1.0)
                recs.append(rec)
                psb = pPsb.tile([128, 1024], MDT, tag="psb", name=f"psb{h}_{comp}")
                nc.vector.tensor_copy(psb, pp)
                psbs.append(psb)

            return boundary

        def emit_comp(h, comp, qs, qw, ks, kn, v2t, recs, psbs, filler=None):
            ks_t = kn if comp == 0 else ks
            qs_t = qs if comp == 0 else qw
            sums = ps_sums.tile([128, 1024], F32, tag="sums", name=f"sums{h}_{comp}")
            pp = ps_p.tile([128, 1024], F32, tag="p", name=f"pp{h}_{comp}")
            p0 = pp[:, 0:512]
            p1 = pp[:, 512:1024]
            e_prev = None
            sums_pend = None
            for tk in range(TC):
                st = ps_pair.tile([128, 1024], F32, tag="pair", name=f"st{h}_{comp}_{tk}")
                ksl = slice(tk * 128, (tk + 1) * 128)
                mm(st[:, 0:512], ks_t[:, ksl], qs_t[:, 0:512], True, True)
                mm(st[:, 512:1024], ks_t[:, ksl], qs_t[:, 512:1024], True, True)
                e = pE.tile([128, 1024], MDT, tag="e", name=f"e{h}_{comp}_{tk}")
                nc.scalar.activation(e, st, func=EXP)
                if tk % 2 == 1:
                    esum = pEs.tile(
                        [128, 1024], MDT, tag="es", name=f"es{h}_{comp}_{tk}"
                    )
                    nc.vector.tensor_add(esum, e_prev, e)
                if tk == 0 and filler is not None:
                    filler()
                if tk == 0 and state["post"] is not None:
                    flush_post()
                else:
                    flush()
                if tk % 2 == 1:
                    # lag-2: emit the PREVIOUS pair's sums matmuls; this
                    # pair's add has two iterations to complete.
                    if sums_pend is not None:
                        emit_sums(sums_pend)
                    sums_pend = (sums, esum, tk == 1, tk == TC - 1)
                if comp == 0:
                    vt = v1[:, tk * 1024 + h * 128 : tk * 1024 + h * 128 + 128]
                else:
                    vt = v2t[:, tk * 128 : (tk + 1) * 128]
                state["pending"] = (p0, p1, e, vt, tk == 0, tk == TC - 1)
                e_prev = e
            state["post"] = (sums_pend, make_boundary(h, comp, sums, pp, recs, psbs))

        def emit_norm(h, recs, psbs):
            osl = slice(h * 1024, (h + 1) * 1024)
            t1 = pEs.tile([128, 1024], MDT, tag="es", name=f"t1_{h}")
            nc.vector.tensor_mul(t1, psbs[0], recs[0])
            nc.vector.tensor_mul(osb[:, osl], psbs[1], recs[1])
            nc.vector.tensor_add(osb[:, osl], osb[:, osl], t1)

        # prologue: head 0 projections
        prefetch_w(0)
        for kk in range(KC):
            tk_ = pB.tile([128, S], MDT, tag="bigB", name=f"xtk{kk}")
            dma_split(tk_, d_xtk[kk], n=2)
            xtk.append(tk_)
        prefetch_w(1)
        for kk in range(KC):
            tq = pA.tile([128, S], MDT, tag="bigA", name=f"xtq{kk}")
            dma_split(tq, d_xtq[kk], n=2)
            xtq.append(tq)
        kp = emit_k_chain(0)
        ks0, kn0 = finish_k(0, kp)
        qp = emit_q_chain(0)
        qs0, qw0 = finish_q(0, qp)
        cur = (qs0, qw0, ks0, kn0, build_v2(0))

        kp_box = {}
        for h in range(H):
            prefetch_w(h + 2)
            recs, psbs = [], []
            emit_comp(h, 0, *cur, recs, psbs)
            # splice the next head's K chain into the comp0->comp1 boundary
            # (PE filler while exp(c0,7) + ln/rec complete)
            filler = None
            if h + 1 < H:
                def filler(hh=h + 1):
                    kp_box["kp"] = emit_k_chain(hh)
            emit_comp(h, 1, *cur, recs, psbs, filler=filler)
            if h + 1 < H:
                ksn, knn = finish_k(h + 1, kp_box.pop("kp"))
                qp = emit_q_chain(h + 1)  # PE filler for the c1-iter7 flush
                flush_post()  # c1 final AV+sums + boundary (rec_i, P2 evac)
                qsn, qwn = finish_q(h + 1, qp)
                nxt = (qsn, qwn, ksn, knn, build_v2(h + 1))
            else:
                # last head: prefetch wo during the tail
                wo_t = []
                for hh in range(H):
                    tw = pA.tile([128, 1024], MDT, tag="bigA", name=f"wo{hh}")
                    nc.sync.dma_start(out=tw, in_=d_wo[hh])
                    wo_t.append(tw)
                flush_post()
                nxt = None
            emit_norm(h, recs, psbs)
            cur = nxt

        # ---- output projection (transposed: out[odim, tok]) ----
        for oc in range(KC):
            pair = ps_pair.tile([128, 1024], F32, tag="pair", name=f"ops{oc}")
            osl = slice(oc * 128, (oc + 1) * 128)
            for hh in range(H):
                lhsT = wo_t[hh][:, osl]
                hb = hh * 1024
                mm(pair[:, 0:512], lhsT, osb[:, hb : hb + 512], hh == 0, hh == H - 1)
                mm(
                    pair[:, 512:1024],
                    lhsT,
                    osb[:, hb + 512 : hb + 1024],
                    hh == 0,
                    hh == H - 1,
                )
            oev = pB.tile([128, 1024], F32, tag="bigB", name=f"oev{oc}")
            if oc % 2 == 0:
                nc.scalar.copy(oev, pair)
            else:
                nc.vector.tensor_copy(oev, pair)
            dma_split(d_out[oc], oev, n=4)

    _split_waits(nc)
    return nc


_NC_CACHE = {}


def kernel(
    queries,
    keys,
    values,
    wq_r,
    wq_i,
    wk_r,
    wk_i,
    wv_r,
    wv_i,
    wo_r,
    wo_i,
    _trace=False,
):
    global LAST_EXEC_NS
    _install_axon_profile_shim()
    _install_tile_drain_patch()
    from concourse.bass_utils import run_bass_kernel_spmd

    scale = 1.0 / np.sqrt(DH)
    WQ = _head_tiles(_build_wqk(np.asarray(wq_r), np.asarray(wq_i), scale))
    WK = _head_tiles(_build_wqk(np.asarray(wk_r), np.asarray(wk_i), 1.0))
    WV = _kchunk_tiles(_build_wqk(np.asarray(wv_r), np.asarray(wv_i), 1.0))
    WO = _kchunk_tiles(_build_wo(np.asarray(wo_r), np.asarray(wo_i)))
    CST = np.zeros((128, 320), np.float32)
    CST[:, 0:128] = 1.0

    queries = np.asarray(queries)
    keys = np.asarray(keys)
    values = np.asarray(values)

    in_maps = []
    for b in range(NCORES):
        in_maps.append(
            {
                "xtq": _xt(queries[b]).reshape(KC, 128, S),
                "xtk": _xt(keys[b]).reshape(KC, 128, S),
                "xtv": _xt(values[b]).reshape(KC, 128, S),
                "wq": WQ,
                "wk": WK,
                "wv": WV,
                "wo": WO,
                "cst": CST,
            }
        )

    if "nc" not in _NC_CACHE:
        _NC_CACHE["nc"] = _build_nc()
    nc = _NC_CACHE["nc"]

    res = run_bass_kernel_spmd(nc, in_maps, list(range(NCORES)), trace=_trace)
    LAST_EXEC_NS = res.exec_time_ns

    out = np.empty((B, S, D, 2), np.float32)
    for b in range(NCORES):
        # res: [oc, odim, tok] -> [tok, oc*128+odim] -> [S, D, 2]
        r = res.results[b]["out"].reshape(1024, S)
        out[b] = r.T.reshape(S, D, 2)
    return out


# revision 12
# speedup vs baseline: 1.0896x; 1.0896x over previous
"""Complex multi-head attention on 8 Trainium2 cores (Bass/Tile).

Sharding: pure data-parallel over batch (B=8 -> 1 batch per core),
weights replicated. No collectives.

v2 redesign (vs baseline at 465us):
  - Every LDWEIGHTS serves 2 matmuls (the two 512-col halves of each
    1024-wide rhs share the stationary operand) so the fp32 weight-load
    (224ns, no FWL for fp32) hides under 2x213ns of streaming.
  - Scores for both halves land in one 2-bank PSUM pair tile and are
    exponentiated with a single [128,1024] activation (amortizes the
    ~300-cycle ACT instruction overhead).
  - Softmax reciprocal moved off the DVE (was 32 x 3.9us of iterative
    divide) to the scalar engine as exp(-ln(sums)); Ln+Exp share one
    ACT table set.
  - P1/P2 evacuated unnormalized; normalization (3 DVE ops/head)
    happens off the critical path during the next head's projections.
  - Lag-1 software pipeline: st pair of iter i+1 is emitted between
    exp(i) and the sums/AV matmuls of iter i, so the PE never waits on
    the scalar engine; next head's K/Q projection chains are spliced
    into the head boundary to cover the last exp + rec latency.
  - O-projection computed transposed (out = [odim, tok]) with
    wo-stationary h-chains so its LDW is also paired; host undoes the
    transpose.
PSUM: st/proj/V/O pair pool [128,1024]x2 (4 banks) + sums pair
[128,1024]x1 (2) + P accumulators [128,512]x2 (2) = 8 banks exactly.
"""

import sys
import types
import numpy as np

B, S, D, H = 8, 1024, 512, 8
DH = D // H
KC = 8  # k-chunks of 128 over (c,d) = 1024
TC = 8  # token chunks of 128
NCORES = 8

LAST_EXEC_NS = None


# ---------------------------------------------------------------- shims
def _install_axon_profile_shim():
    if "antenv.axon_hooks" in sys.modules:
        return
    try:
        import antenv  # noqa: F401

        mod = types.ModuleType("antenv.axon_hooks")
        state = {"hook": None}
        mod.set_axon_ntff_profile_hook = lambda h: state.__setitem__("hook", h)
        mod.get_axon_ntff_profile_hook = lambda: state["hook"]
        sys.modules["antenv.axon_hooks"] = mod
        from trn_agent_boot.trn_boot import _ntff_profile_via_ctypes

        hook = _ntff_profile_via_ctypes("/opt/axon/libaxon_pjrt.so")
        if hook is not None:
            mod.set_axon_ntff_profile_hook(hook)
    except Exception:
        pass


def _install_tile_drain_patch():
    """This walrus build allows ONE sync wait per instruction; split the
    TileContext exit drain's waits across preceding sync NOPs."""
    import concourse.mybir as mybir
    import concourse.tile as tile
    from concourse.vector_clock import ScopedClock

    if getattr(tile.TileContext, "_drain_patched", False):
        return

    def _patched(self, tick_clock, wait_clock):
        probe = mybir.InstNoOp(name="I-drain-probe")
        probe.engine = mybir.EngineType.SP
        wait_clock.add_sem_waits(probe, ScopedClock({None: tick_clock.global_clock}))
        waits = list(probe.sync_info.on_wait or []) if probe.sync_info else []
        for w in waits:
            nop = self.nc.sync.nop()
            nop.ins.sync_info = mybir.SyncInfo(on_wait=[w], on_update=[])
        self.nc.sync.drain()
        self.nc.all_engine_barrier()
        assert self.sems is not None
        popped = self.nc._tile_sem_poison_stack.pop()
        assert popped is self._sem_poison
        self.nc.clear_and_free_semaphores(list(self.sems.allocated().values()))
        self.nc.all_engine_barrier()

    tile.TileContext._drain_and_barrier = _patched
    tile.TileContext._drain_patched = True


def _split_waits(nc, max_waits=1):
    """Hoist extra sync waits onto preceding same-engine NOPs (walrus here
    rejects >1 sync wait per instruction)."""
    import concourse.mybir as mybir

    def process(blk):
        lst = blk.instructions
        i = 0
        while i < len(lst):
            inst = lst[i]
            if hasattr(inst, "blocks"):
                for b in inst.blocks or []:
                    process(b)
            si = inst.sync_info
            if si is not None and si.on_wait and len(si.on_wait) > max_waits:
                waits = list(si.on_wait)
                keep, extra = waits[-max_waits:], waits[:-max_waits]
                inst.sync_info = mybir.SyncInfo(
                    on_wait=keep, on_update=list(si.on_update or [])
                )
                for j, w in enumerate(extra):
                    nop = mybir.InstNoOp(name=f"{inst.name}-ws{j}")
                    nop.engine = inst.engine
                    nop.sync_info = mybir.SyncInfo(on_wait=[w], on_update=[])
                    lst.insert(i, nop)
                    i += 1
            i += 1

    for f in nc.m.functions:
        for blk in f.blocks:
            process(blk)


# ------------------------------------------------------------ host prep
def _build_wqk(wr, wi, scale):
    """[1024 k=(c,d), 1024 m=(h, c', dh)] for Q/K projections."""
    W = np.empty((2 * D, 2 * D), np.float32)
    for h in range(H):
        o = slice(h * DH, (h + 1) * DH)
        c0 = h * 2 * DH
        W[0:D, c0 : c0 + DH] = wr[o].T * scale
        W[D:, c0 : c0 + DH] = -wi[o].T * scale
        W[0:D, c0 + DH : c0 + 2 * DH] = wi[o].T * scale
        W[D:, c0 + DH : c0 + 2 * DH] = wr[o].T * scale
    return W


def _head_tiles(W):
    """[1024,1024] -> [H, 128, 1024]: per-head column block, k-chunk cols."""
    out = np.empty((H, 128, 1024), np.float32)
    for h in range(H):
        blk = W[:, h * 128 : (h + 1) * 128]  # [1024, 128]
        for kk in range(KC):
            out[h, :, kk * 128 : (kk + 1) * 128] = blk[kk * 128 : (kk + 1) * 128]
    return out


def _kchunk_tiles(W):
    """[1024,1024] -> [KC, 128, 1024]: row chunks."""
    return np.ascontiguousarray(W.reshape(KC, 128, 1024))


def _build_wo(wo_r, wo_i):
    """rows (h, c', dh), cols (o, c) interleaved to match [S, D, 2]."""
    W = np.empty((2 * D, 2 * D), np.float32)
    for h in range(H):
        d = slice(h * DH, (h + 1) * DH)
        r0 = h * 2 * DH
        W[r0 : r0 + DH, 0::2] = wo_r[:, d].T
        W[r0 : r0 + DH, 1::2] = wo_i[:, d].T
        W[r0 + DH : r0 + 2 * DH, 0::2] = -wo_i[:, d].T
        W[r0 + DH : r0 + 2 * DH, 1::2] = wo_r[:, d].T
    return W


def _xt(x):  # [S, D, 2] -> [2D, S] feature-major
    out = np.empty((2 * D, S), np.float32)
    out[0:D] = x[:, :, 0].T
    out[D:] = x[:, :, 1].T
    return out


# ------------------------------------------------------------ bass build
def _build_nc():
    import concourse.bass as bass
    import concourse.mybir as mybir
    import concourse.tile as tile
    from contextlib import ExitStack

    MDT = mybir.dt.float32r
    F32 = mybir.dt.float32
    EXP = mybir.ActivationFunctionType.Exp
    LN = mybir.ActivationFunctionType.Ln

    nc = bass.Bass()
    d_xtq = nc.dram_tensor("xtq", [KC, 128, S], MDT, kind="ExternalInput")
    d_xtk = nc.dram_tensor("xtk", [KC, 128, S], MDT, kind="ExternalInput")
    d_xtv = nc.dram_tensor("xtv", [KC, 128, S], MDT, kind="ExternalInput")
    d_wq = nc.dram_tensor("wq", [H, 128, 1024], MDT, kind="ExternalInput")
    d_wk = nc.dram_tensor("wk", [H, 128, 1024], MDT, kind="ExternalInput")
    d_wv = nc.dram_tensor("wv", [KC, 128, 1024], MDT, kind="ExternalInput")
    d_wo = nc.dram_tensor("wo", [H, 128, 1024], MDT, kind="ExternalInput")
    d_cst = nc.dram_tensor("cst", [128, 320], MDT, kind="ExternalInput")
    # transposed output: [oc, odim, tok]
    d_out = nc.dram_tensor("out", [KC, 128, S], F32, kind="ExternalOutput")

    with tile.TileContext(nc) as tc, ExitStack() as ctx:
        ctx.enter_context(
            nc.allow_low_precision(reason="float32r tiles are bit-identical fp32")
        )
        pA = ctx.enter_context(tc.tile_pool(name="bigA", bufs=8))  # xtv->xtq->wo
        pB = ctx.enter_context(tc.tile_pool(name="bigB", bufs=8))  # wv->xtk->oev
        pV1 = ctx.enter_context(tc.tile_pool(name="v1", bufs=1))
        pOsb = ctx.enter_context(tc.tile_pool(name="osb", bufs=1))
        pQs = ctx.enter_context(tc.tile_pool(name="qs", bufs=2))
        pKs = ctx.enter_context(tc.tile_pool(name="ks", bufs=2))
        pQw = ctx.enter_context(tc.tile_pool(name="qw", bufs=1))
        pKn = ctx.enter_context(tc.tile_pool(name="kn", bufs=1))
        pV2 = ctx.enter_context(tc.tile_pool(name="v2", bufs=1))
        pE = ctx.enter_context(tc.tile_pool(name="e", bufs=2))
        pEs = ctx.enter_context(tc.tile_pool(name="es", bufs=2))
        pPsb = ctx.enter_context(tc.tile_pool(name="psb", bufs=2))
        pRec = ctx.enter_context(tc.tile_pool(name="rec", bufs=3))
        pWqk = ctx.enter_context(tc.tile_pool(name="wqk", bufs=3))
        pC = ctx.enter_context(tc.tile_pool(name="const", bufs=1))

        ps_pair = ctx.enter_context(tc.tile_pool(name="ps_pair", bufs=2, space="PSUM"))
        ps_sums = ctx.enter_context(tc.tile_pool(name="ps_sums", bufs=1, space="PSUM"))
        ps_p = ctx.enter_context(tc.tile_pool(name="ps_p", bufs=1, space="PSUM"))

        cst = pC.tile([128, 320], MDT, tag="cst")
        nc.sync.dma_start(out=cst, in_=d_cst[:, :])
        ones128 = cst[:, 0:128]

        def mm(out, lhsT, rhs, start, stop):
            nc.tensor.matmul(out, lhsT=lhsT, rhs=rhs, start=start, stop=stop)

        # ---- phase V: V projection (all heads) ----
        def dma_split(out, in_, n=4):
            w = out.shape[-1] // n
            for i in range(n):
                nc.sync.dma_start(
                    out=out[:, i * w : (i + 1) * w], in_=in_[:, i * w : (i + 1) * w]
                )

        xtv, wv = [], []
        for kk in range(KC):
            tv = pA.tile([128, S], MDT, tag="bigA", name=f"xtv{kk}")
            tw = pB.tile([128, 1024], MDT, tag="bigB", name=f"wv{kk}")
            if kk < 2:
                dma_split(tv, d_xtv[kk], n=2)
                dma_split(tw, d_wv[kk], n=2)
            else:
                nc.sync.dma_start(out=tv, in_=d_xtv[kk])
                nc.sync.dma_start(out=tw, in_=d_wv[kk])
            xtv.append(tv)
            wv.append(tw)

        v1 = pV1.tile([128, TC * 1024], MDT, tag="v1")  # [p, (t_, h, dh2)]
        for t_ in range(TC):
            pair = ps_pair.tile([128, 1024], F32, tag="pair", name=f"vps{t_}")
            tsl = slice(t_ * 128, (t_ + 1) * 128)
            for kk in range(KC):
                lhsT = xtv[kk][:, tsl]
                mm(pair[:, 0:512], lhsT, wv[kk][:, 0:512], kk == 0, kk == KC - 1)
                mm(pair[:, 512:1024], lhsT, wv[kk][:, 512:1024], kk == 0, kk == KC - 1)
            nc.vector.tensor_copy(v1[:, t_ * 1024 : (t_ + 1) * 1024], pair)

        # ---- load XT_q / XT_k (reuse pA / pB slots) ----
        xtq, xtk = [], []

        osb = pOsb.tile([128, H * 1024], MDT, tag="osb")  # [p, (h, tok)]

        # ---------------- per-head helpers ----------------
        wq_t, wk_t = {}, {}

        def prefetch_w(h):
            if h >= H or h in wq_t:
                return
            tk2 = pWqk.tile([128, 1024], MDT, tag="wqk", name=f"wk{h}")
            dma_split(tk2, d_wk[h], n=2)
            wk_t[h] = tk2
            tq = pWqk.tile([128, 1024], MDT, tag="wqk", name=f"wq{h}")
            dma_split(tq, d_wq[h], n=2)
            wq_t[h] = tq

        def proj_chain(w_tile, xt_tiles, name):
            pair = ps_pair.tile([128, 1024], F32, tag="pair", name=name)
            for kk in range(KC):
                lhsT = w_tile[:, kk * 128 : (kk + 1) * 128]
                mm(pair[:, 0:512], lhsT, xt_tiles[kk][:, 0:512], kk == 0, kk == KC - 1)
                mm(
                    pair[:, 512:1024],
                    lhsT,
                    xt_tiles[kk][:, 512:1024],
                    kk == 0,
                    kk == KC - 1,
                )
            return pair

        def emit_k_chain(h):
            return proj_chain(wk_t.pop(h), xtk, f"kproj{h}")

        def finish_k(h, pair):
            ks = pKs.tile([128, S], MDT, tag="ks", name=f"kstack{h}")
            nc.vector.tensor_copy(ks, pair)
            kn = pKn.tile([128, S], MDT, tag="kn", name=f"kneg{h}")
            nc.vector.tensor_copy(kn[0:64, :], ks[0:64, :])
            nc.vector.tensor_scalar_mul(kn[64:128, :], ks[64:128, :], -1.0)
            return ks, kn

        def emit_q_chain(h):
            return proj_chain(wq_t.pop(h), xtq, f"qproj{h}")

        def finish_q(h, pair):
            qs = pQs.tile([128, S], MDT, tag="qs", name=f"qstack{h}")
            nc.vector.tensor_copy(qs[:, 0:512], pair[:, 0:512])
            nc.vector.tensor_copy(qs[:, 512:1024], pair[:, 512:1024])
            qw = pQw.tile([128, S], MDT, tag="qw", name=f"qswap{h}")
            nc.sync.dma_start(out=qw[0:64, :], in_=qs[64:128, :])
            nc.sync.dma_start(out=qw[64:128, :], in_=qs[0:64, :])
            return qs, qw

        def build_v2(h):
            v2t = pV2.tile([128, 1024], MDT, tag="v2", name=f"v2h{h}")
            v1v = v1.rearrange("p (t h d) -> p t h d", t=TC, h=H, d=128)
            v2v = v2t.rearrange("p (t d) -> p t d", t=TC, d=128)
            nc.vector.tensor_scalar_mul(
                v2v[:, :, 0:64], v1v[:, :, h, 64:128], -1.0
            )
            nc.vector.tensor_copy(v2v[:, :, 64:128], v1v[:, :, h, 0:64])
            return v2t

        # ---------------- the pipelined head loop ----------------
        # pending = (sums, p0, p1, e, vt, first, last, boundary_cb)
        state = {"pending": None, "post": None}

        def flush_post():
            flush()
            boundary = state["post"]
            boundary()
            state["post"] = None

        def flush():
            p = state["pending"]
            if p is None:
                return
            sums, p0, p1, e, vt, first, last = p
            mm(sums[:, 0:512], ones128, e[:, 0:512], first, last)
            mm(sums[:, 512:1024], ones128, e[:, 512:1024], first, last)
            mm(p0, vt, e[:, 0:512], first, last)
            mm(p1, vt, e[:, 512:1024], first, last)
            state["pending"] = None

        def make_boundary(h, comp, sums, pp, recs, psbs):
            def boundary():
                lnt = pRec.tile([128, 1024], MDT, tag="rec", name=f"lnt{h}_{comp}")
                nc.scalar.activation(lnt, sums, func=LN)
                rec = pRec.tile([128, 1024], MDT, tag="rec", name=f"rec{h}_{comp}")
                nc.scalar.activation(rec, lnt, func=EXP, scale=-1.0)
                recs.append(rec)
                psb = pPsb.tile([128, 1024], MDT, tag="psb", name=f"psb{h}_{comp}")
                nc.vector.tensor_copy(psb, pp)
                psbs.append(psb)

            return boundary

        def emit_comp(h, comp, qs, qw, ks, kn, v2t, recs, psbs, filler=None):
            ks_t = kn if comp == 0 else ks
            qs_t = qs if comp == 0 else qw
            sums = ps_sums.tile([128, 1024], F32, tag="sums", name=f"sums{h}_{comp}")
            pp = ps_p.tile([128, 1024], F32, tag="p", name=f"pp{h}_{comp}")
            p0 = pp[:, 0:512]
            p1 = pp[:, 512:1024]
            for tk in range(TC):
                st = ps_pair.tile([128, 1024], F32, tag="pair", name=f"st{h}_{comp}_{tk}")
                ksl = slice(tk * 128, (tk + 1) * 128)
                mm(st[:, 0:512], ks_t[:, ksl], qs_t[:, 0:512], True, True)
                mm(st[:, 512:1024], ks_t[:, ksl], qs_t[:, 512:1024], True, True)
                e = pE.tile([128, 1024], MDT, tag="e", name=f"e{h}_{comp}_{tk}")
                nc.scalar.activation(e, st, func=EXP)
                if tk == 0 and filler is not None:
                    filler()
                if tk == 0 and state["post"] is not None:
                    flush_post()
                else:
                    flush()
                if comp == 0:
                    vt = v1[:, tk * 1024 + h * 128 : tk * 1024 + h * 128 + 128]
                else:
                    vt = v2t[:, tk * 128 : (tk + 1) * 128]
                state["pending"] = (sums, p0, p1, e, vt, tk == 0, tk == TC - 1)
            state["post"] = make_boundary(h, comp, sums, pp, recs, psbs)

        def emit_norm(h, recs, psbs):
            osl = slice(h * 1024, (h + 1) * 1024)
            t1 = pEs.tile([128, 1024], MDT, tag="es", name=f"t1_{h}")
            nc.vector.tensor_mul(t1, psbs[0], recs[0])
            nc.vector.tensor_mul(osb[:, osl], psbs[1], recs[1])
            nc.vector.tensor_add(osb[:, osl], osb[:, osl], t1)

        # prologue: head 0 projections
        prefetch_w(0)
        for kk in range(KC):
            tk_ = pB.tile([128, S], MDT, tag="bigB", name=f"xtk{kk}")
            dma_split(tk_, d_xtk[kk], n=2)
            xtk.append(tk_)
        prefetch_w(1)
        for kk in range(KC):
            tq = pA.tile([128, S], MDT, tag="bigA", name=f"xtq{kk}")
            dma_split(tq, d_xtq[kk], n=2)
            xtq.append(tq)
        kp = emit_k_chain(0)
        ks0, kn0 = finish_k(0, kp)
        qp = emit_q_chain(0)
        qs0, qw0 = finish_q(0, qp)
        cur = (qs0, qw0, ks0, kn0, build_v2(0))

        kp_box = {}
        for h in range(H):
            prefetch_w(h + 2)
            recs, psbs = [], []
            emit_comp(h, 0, *cur, recs, psbs)
            # splice the next head's K chain into the comp0->comp1 boundary
            # (PE filler while exp(c0,7) + ln/rec complete)
            filler = None
            if h + 1 < H:
                def filler(hh=h + 1):
                    kp_box["kp"] = emit_k_chain(hh)
            emit_comp(h, 1, *cur, recs, psbs, filler=filler)
            if h + 1 < H:
                ksn, knn = finish_k(h + 1, kp_box.pop("kp"))
                qp = emit_q_chain(h + 1)  # PE filler for the c1-iter7 flush
                flush_post()  # c1 final AV+sums + boundary (rec_i, P2 evac)
                qsn, qwn = finish_q(h + 1, qp)
                nxt = (qsn, qwn, ksn, knn, build_v2(h + 1))
            else:
                # last head: prefetch wo during the tail
                wo_t = []
                for hh in range(H):
                    tw = pA.tile([128, 1024], MDT, tag="bigA", name=f"wo{hh}")
                    nc.sync.dma_start(out=tw, in_=d_wo[hh])
                    wo_t.append(tw)
                flush_post()
                nxt = None
            emit_norm(h, recs, psbs)
            cur = nxt

        # ---- output projection (transposed: out[odim, tok]) ----
        for oc in range(KC):
            pair = ps_pair.tile([128, 1024], F32, tag="pair", name=f"ops{oc}")
            osl = slice(oc * 128, (oc + 1) * 128)
            for hh in range(H):
                lhsT = wo_t[hh][:, osl]
                hb = hh * 1024
                mm(pair[:, 0:512], lhsT, osb[:, hb : hb + 512], hh == 0, hh == H - 1)
                mm(
                    pair[:, 512:1024],
                    lhsT,
                    osb[:, hb + 512 : hb + 1024],
                    hh == 0,
                    hh == H - 1,
                )
            oev = pB.tile([128, 1024], F32, tag="bigB", name=f"oev{oc}")
            if oc % 2 == 0:
                nc.scalar.copy(oev, pair)
            else:
                nc.vector.tensor_copy(oev, pair)
            dma_split(d_out[oc], oev, n=4)

    _split_waits(nc)
    return nc


_NC_CACHE = {}


def kernel(
    queries,
    keys,
    values,
    wq_r,
    wq_i,
    wk_r,
    wk_i,
    wv_r,
    wv_i,
    wo_r,
    wo_i,
    _trace=False,
):
    global LAST_EXEC_NS
    _install_axon_profile_shim()
    _install_tile_drain_patch()
    from concourse.bass_utils import run_bass_kernel_spmd

    scale = 1.0 / np.sqrt(DH)
    WQ = _head_tiles(_build_wqk(np.asarray(wq_r), np.asarray(wq_i), scale))
    WK = _head_tiles(_build_wqk(np.asarray(wk_r), np.asarray(wk_i), 1.0))
    WV = _kchunk_tiles(_build_wqk(np.asarray(wv_r), np.asarray(wv_i), 1.0))
    WO = _kchunk_tiles(_build_wo(np.asarray(wo_r), np.asarray(wo_i)))
    CST = np.zeros((128, 320), np.float32)
    CST[:, 0:128] = 1.0

    queries = np.asarray(queries)
    keys = np.asarray(keys)
    values = np.asarray(values)

    in_maps = []
    for b in range(NCORES):
        in_maps.append(
            {
                "xtq": _xt(queries[b]).reshape(KC, 128, S),
                "xtk": _xt(keys[b]).reshape(KC, 128, S),
                "xtv": _xt(values[b]).reshape(KC, 128, S),
                "wq": WQ,
                "wk": WK,
                "wv": WV,
                "wo": WO,
                "cst": CST,
            }
        )

    if "nc" not in _NC_CACHE:
        _NC_CACHE["nc"] = _build_nc()
    nc = _NC_CACHE["nc"]

    res = run_bass_kernel_spmd(nc, in_maps, list(range(NCORES)), trace=_trace)
    LAST_EXEC_NS = res.exec_time_ns

    out = np.empty((B, S, D, 2), np.float32)
    for b in range(NCORES):
        # res: [oc, odim, tok] -> [tok, oc*128+odim] -> [S, D, 2]
        r = res.results[b]["out"].reshape(1024, S)
        out[b] = r.T.reshape(S, D, 2)
    return out


# revision 13
# speedup vs baseline: 1.0954x; 1.0053x over previous
"""Complex multi-head attention on 8 Trainium2 cores (Bass/Tile).

Sharding: pure data-parallel over batch (B=8 -> 1 batch per core),
weights replicated. No collectives.

v2 redesign (vs baseline at 465us):
  - Every LDWEIGHTS serves 2 matmuls (the two 512-col halves of each
    1024-wide rhs share the stationary operand) so the fp32 weight-load
    (224ns, no FWL for fp32) hides under 2x213ns of streaming.
  - Scores for both halves land in one 2-bank PSUM pair tile and are
    exponentiated with a single [128,1024] activation (amortizes the
    ~300-cycle ACT instruction overhead).
  - Softmax reciprocal moved off the DVE (was 32 x 3.9us of iterative
    divide) to the scalar engine as exp(-ln(sums)); Ln+Exp share one
    ACT table set.
  - P1/P2 evacuated unnormalized; normalization (3 DVE ops/head)
    happens off the critical path during the next head's projections.
  - Lag-1 software pipeline: st pair of iter i+1 is emitted between
    exp(i) and the sums/AV matmuls of iter i, so the PE never waits on
    the scalar engine; next head's K/Q projection chains are spliced
    into the head boundary to cover the last exp + rec latency.
  - O-projection computed transposed (out = [odim, tok]) with
    wo-stationary h-chains so its LDW is also paired; host undoes the
    transpose.
PSUM: st/proj/V/O pair pool [128,1024]x2 (4 banks) + sums pair
[128,1024]x1 (2) + P accumulators [128,512]x2 (2) = 8 banks exactly.
"""

import sys
import types
import numpy as np

B, S, D, H = 8, 1024, 512, 8
DH = D // H
KC = 8  # k-chunks of 128 over (c,d) = 1024
TC = 8  # token chunks of 128
NCORES = 8

LAST_EXEC_NS = None


# ---------------------------------------------------------------- shims
def _install_axon_profile_shim():
    if "antenv.axon_hooks" in sys.modules:
        return
    try:
        import antenv  # noqa: F401

        mod = types.ModuleType("antenv.axon_hooks")
        state = {"hook": None}
        mod.set_axon_ntff_profile_hook = lambda h: state.__setitem__("hook", h)
        mod.get_axon_ntff_profile_hook = lambda: state["hook"]
        sys.modules["antenv.axon_hooks"] = mod
        from trn_agent_boot.trn_boot import _ntff_profile_via_ctypes

        hook = _ntff_profile_via_ctypes("/opt/axon/libaxon_pjrt.so")
        if hook is not None:
            mod.set_axon_ntff_profile_hook(hook)
    except Exception:
        pass


def _install_tile_drain_patch():
    """This walrus build allows ONE sync wait per instruction; split the
    TileContext exit drain's waits across preceding sync NOPs."""
    import concourse.mybir as mybir
    import concourse.tile as tile
    from concourse.vector_clock import ScopedClock

    if getattr(tile.TileContext, "_drain_patched", False):
        return

    def _patched(self, tick_clock, wait_clock):
        probe = mybir.InstNoOp(name="I-drain-probe")
        probe.engine = mybir.EngineType.SP
        wait_clock.add_sem_waits(probe, ScopedClock({None: tick_clock.global_clock}))
        waits = list(probe.sync_info.on_wait or []) if probe.sync_info else []
        for w in waits:
            nop = self.nc.sync.nop()
            nop.ins.sync_info = mybir.SyncInfo(on_wait=[w], on_update=[])
        self.nc.sync.drain()
        self.nc.all_engine_barrier()
        assert self.sems is not None
        popped = self.nc._tile_sem_poison_stack.pop()
        assert popped is self._sem_poison
        self.nc.clear_and_free_semaphores(list(self.sems.allocated().values()))
        self.nc.all_engine_barrier()

    tile.TileContext._drain_and_barrier = _patched
    tile.TileContext._drain_patched = True


def _split_waits(nc, max_waits=1):
    """Hoist extra sync waits onto preceding same-engine NOPs (walrus here
    rejects >1 sync wait per instruction)."""
    import concourse.mybir as mybir

    def process(blk):
        lst = blk.instructions
        i = 0
        while i < len(lst):
            inst = lst[i]
            if hasattr(inst, "blocks"):
                for b in inst.blocks or []:
                    process(b)
            si = inst.sync_info
            if si is not None and si.on_wait and len(si.on_wait) > max_waits:
                waits = list(si.on_wait)
                keep, extra = waits[-max_waits:], waits[:-max_waits]
                inst.sync_info = mybir.SyncInfo(
                    on_wait=keep, on_update=list(si.on_update or [])
                )
                for j, w in enumerate(extra):
                    nop = mybir.InstNoOp(name=f"{inst.name}-ws{j}")
                    nop.engine = inst.engine
                    nop.sync_info = mybir.SyncInfo(on_wait=[w], on_update=[])
                    lst.insert(i, nop)
                    i += 1
            i += 1

    for f in nc.m.functions:
        for blk in f.blocks:
            process(blk)


# ------------------------------------------------------------ host prep
def _build_wqk(wr, wi, scale):
    """[1024 k=(c,d), 1024 m=(h, c', dh)] for Q/K projections."""
    W = np.empty((2 * D, 2 * D), np.float32)
    for h in range(H):
        o = slice(h * DH, (h + 1) * DH)
        c0 = h * 2 * DH
        W[0:D, c0 : c0 + DH] = wr[o].T * scale
        W[D:, c0 : c0 + DH] = -wi[o].T * scale
        W[0:D, c0 + DH : c0 + 2 * DH] = wi[o].T * scale
        W[D:, c0 + DH : c0 + 2 * DH] = wr[o].T * scale
    return W


def _head_tiles(W):
    """[1024,1024] -> [H, 128, 1024]: per-head column block, k-chunk cols."""
    out = np.empty((H, 128, 1024), np.float32)
    for h in range(H):
        blk = W[:, h * 128 : (h + 1) * 128]  # [1024, 128]
        for kk in range(KC):
            out[h, :, kk * 128 : (kk + 1) * 128] = blk[kk * 128 : (kk + 1) * 128]
    return out


def _kchunk_tiles(W):
    """[1024,1024] -> [KC, 128, 1024]: row chunks."""
    return np.ascontiguousarray(W.reshape(KC, 128, 1024))


def _build_wo(wo_r, wo_i):
    """rows (h, c', dh), cols (o, c) interleaved to match [S, D, 2]."""
    W = np.empty((2 * D, 2 * D), np.float32)
    for h in range(H):
        d = slice(h * DH, (h + 1) * DH)
        r0 = h * 2 * DH
        W[r0 : r0 + DH, 0::2] = wo_r[:, d].T
        W[r0 : r0 + DH, 1::2] = wo_i[:, d].T
        W[r0 + DH : r0 + 2 * DH, 0::2] = -wo_i[:, d].T
        W[r0 + DH : r0 + 2 * DH, 1::2] = wo_r[:, d].T
    return W


def _xt(x):  # [S, D, 2] -> [2D, S] feature-major
    out = np.empty((2 * D, S), np.float32)
    out[0:D] = x[:, :, 0].T
    out[D:] = x[:, :, 1].T
    return out


# ------------------------------------------------------------ bass build
def _build_nc():
    import concourse.bass as bass
    import concourse.mybir as mybir
    import concourse.tile as tile
    from contextlib import ExitStack

    MDT = mybir.dt.float32r
    F32 = mybir.dt.float32
    EXP = mybir.ActivationFunctionType.Exp
    LN = mybir.ActivationFunctionType.Ln

    nc = bass.Bass()
    d_xtq = nc.dram_tensor("xtq", [KC, 128, S], MDT, kind="ExternalInput")
    d_xtk = nc.dram_tensor("xtk", [KC, 128, S], MDT, kind="ExternalInput")
    d_xtv = nc.dram_tensor("xtv", [KC, 128, S], MDT, kind="ExternalInput")
    d_wq = nc.dram_tensor("wq", [H, 128, 1024], MDT, kind="ExternalInput")
    d_wk = nc.dram_tensor("wk", [H, 128, 1024], MDT, kind="ExternalInput")
    d_wv = nc.dram_tensor("wv", [KC, 128, 1024], MDT, kind="ExternalInput")
    d_wo = nc.dram_tensor("wo", [H, 128, 1024], MDT, kind="ExternalInput")
    d_cst = nc.dram_tensor("cst", [128, 320], MDT, kind="ExternalInput")
    # transposed output: [oc, odim, tok]
    d_out = nc.dram_tensor("out", [KC, 128, S], F32, kind="ExternalOutput")

    with tile.TileContext(nc) as tc, ExitStack() as ctx:
        ctx.enter_context(
            nc.allow_low_precision(reason="float32r tiles are bit-identical fp32")
        )
        pA = ctx.enter_context(tc.tile_pool(name="bigA", bufs=8))  # xtv->xtq->wo
        pB = ctx.enter_context(tc.tile_pool(name="bigB", bufs=8))  # wv->xtk->oev
        pV1 = ctx.enter_context(tc.tile_pool(name="v1", bufs=1))
        pOsb = ctx.enter_context(tc.tile_pool(name="osb", bufs=1))
        pQs = ctx.enter_context(tc.tile_pool(name="qs", bufs=2))
        pKs = ctx.enter_context(tc.tile_pool(name="ks", bufs=2))
        pQw = ctx.enter_context(tc.tile_pool(name="qw", bufs=1))
        pKn = ctx.enter_context(tc.tile_pool(name="kn", bufs=1))
        pV2 = ctx.enter_context(tc.tile_pool(name="v2", bufs=1))
        pE = ctx.enter_context(tc.tile_pool(name="e", bufs=2))
        pEs = ctx.enter_context(tc.tile_pool(name="es", bufs=2))
        pPsb = ctx.enter_context(tc.tile_pool(name="psb", bufs=2))
        pRec = ctx.enter_context(tc.tile_pool(name="rec", bufs=3))
        pWqk = ctx.enter_context(tc.tile_pool(name="wqk", bufs=3))
        pC = ctx.enter_context(tc.tile_pool(name="const", bufs=1))

        ps_pair = ctx.enter_context(tc.tile_pool(name="ps_pair", bufs=2, space="PSUM"))
        ps_sums = ctx.enter_context(tc.tile_pool(name="ps_sums", bufs=1, space="PSUM"))
        ps_p = ctx.enter_context(tc.tile_pool(name="ps_p", bufs=1, space="PSUM"))

        cst = pC.tile([128, 320], MDT, tag="cst")
        nc.sync.dma_start(out=cst, in_=d_cst[:, :])
        ones128 = cst[:, 0:128]

        def mm(out, lhsT, rhs, start, stop):
            nc.tensor.matmul(out, lhsT=lhsT, rhs=rhs, start=start, stop=stop)

        # ---- phase V: V projection (all heads) ----
        def dma_split(out, in_, n=4):
            w = out.shape[-1] // n
            for i in range(n):
                nc.sync.dma_start(
                    out=out[:, i * w : (i + 1) * w], in_=in_[:, i * w : (i + 1) * w]
                )

        xtv, wv = [], []
        for kk in range(KC):
            tv = pA.tile([128, S], MDT, tag="bigA", name=f"xtv{kk}")
            nc.sync.dma_start(out=tv, in_=d_xtv[kk])
            xtv.append(tv)
            tw = pB.tile([128, 1024], MDT, tag="bigB", name=f"wv{kk}")
            nc.sync.dma_start(out=tw, in_=d_wv[kk])
            wv.append(tw)

        v1 = pV1.tile([128, TC * 1024], MDT, tag="v1")  # [p, (t_, h, dh2)]
        for t_ in range(TC):
            pair = ps_pair.tile([128, 1024], F32, tag="pair", name=f"vps{t_}")
            tsl = slice(t_ * 128, (t_ + 1) * 128)
            for kk in range(KC):
                lhsT = xtv[kk][:, tsl]
                mm(pair[:, 0:512], lhsT, wv[kk][:, 0:512], kk == 0, kk == KC - 1)
                mm(pair[:, 512:1024], lhsT, wv[kk][:, 512:1024], kk == 0, kk == KC - 1)
            nc.vector.tensor_copy(v1[:, t_ * 1024 : (t_ + 1) * 1024], pair)

        # ---- load XT_q / XT_k (reuse pA / pB slots) ----
        xtq, xtk = [], []

        osb = pOsb.tile([128, H * 1024], MDT, tag="osb")  # [p, (h, tok)]

        # ---------------- per-head helpers ----------------
        wq_t, wk_t = {}, {}

        def prefetch_w(h):
            if h >= H or h in wq_t:
                return
            tk2 = pWqk.tile([128, 1024], MDT, tag="wqk", name=f"wk{h}")
            dma_split(tk2, d_wk[h], n=2)
            wk_t[h] = tk2
            tq = pWqk.tile([128, 1024], MDT, tag="wqk", name=f"wq{h}")
            dma_split(tq, d_wq[h], n=2)
            wq_t[h] = tq

        def proj_chain(w_tile, xt_tiles, name):
            pair = ps_pair.tile([128, 1024], F32, tag="pair", name=name)
            for kk in range(KC):
                lhsT = w_tile[:, kk * 128 : (kk + 1) * 128]
                mm(pair[:, 0:512], lhsT, xt_tiles[kk][:, 0:512], kk == 0, kk == KC - 1)
                mm(
                    pair[:, 512:1024],
                    lhsT,
                    xt_tiles[kk][:, 512:1024],
                    kk == 0,
                    kk == KC - 1,
                )
            return pair

        def emit_k_chain(h):
            return proj_chain(wk_t.pop(h), xtk, f"kproj{h}")

        def finish_k(h, pair):
            ks = pKs.tile([128, S], MDT, tag="ks", name=f"kstack{h}")
            nc.vector.tensor_copy(ks, pair)
            kn = pKn.tile([128, S], MDT, tag="kn", name=f"kneg{h}")
            nc.vector.tensor_copy(kn[0:64, :], ks[0:64, :])
            nc.vector.tensor_scalar_mul(kn[64:128, :], ks[64:128, :], -1.0)
            return ks, kn

        def emit_q_chain(h):
            return proj_chain(wq_t.pop(h), xtq, f"qproj{h}")

        def finish_q(h, pair):
            qs = pQs.tile([128, S], MDT, tag="qs", name=f"qstack{h}")
            nc.vector.tensor_copy(qs[:, 0:512], pair[:, 0:512])
            nc.vector.tensor_copy(qs[:, 512:1024], pair[:, 512:1024])
            qw = pQw.tile([128, S], MDT, tag="qw", name=f"qswap{h}")
            nc.sync.dma_start(out=qw[0:64, :], in_=qs[64:128, :])
            nc.sync.dma_start(out=qw[64:128, :], in_=qs[0:64, :])
            return qs, qw

        def build_v2(h):
            v2t = pV2.tile([128, 1024], MDT, tag="v2", name=f"v2h{h}")
            v1v = v1.rearrange("p (t h d) -> p t h d", t=TC, h=H, d=128)
            v2v = v2t.rearrange("p (t d) -> p t d", t=TC, d=128)
            nc.vector.tensor_scalar_mul(
                v2v[:, :, 0:64], v1v[:, :, h, 64:128], -1.0
            )
            nc.vector.tensor_copy(v2v[:, :, 64:128], v1v[:, :, h, 0:64])
            return v2t

        # ---------------- the pipelined head loop ----------------
        # pending = (sums, p0, p1, e, vt, first, last, boundary_cb)
        state = {"pending": None, "post": None}

        def flush_post():
            flush()
            boundary = state["post"]
            boundary()
            state["post"] = None

        def flush():
            p = state["pending"]
            if p is None:
                return
            sums, p0, p1, e, vt, first, last = p
            mm(sums[:, 0:512], ones128, e[:, 0:512], first, last)
            mm(sums[:, 512:1024], ones128, e[:, 512:1024], first, last)
            mm(p0, vt, e[:, 0:512], first, last)
            mm(p1, vt, e[:, 512:1024], first, last)
            state["pending"] = None

        def make_boundary(h, comp, sums, pp, recs, psbs):
            def boundary():
                lnt = pRec.tile([128, 1024], MDT, tag="rec", name=f"lnt{h}_{comp}")
                nc.scalar.activation(lnt, sums, func=LN)
                rec = pRec.tile([128, 1024], MDT, tag="rec", name=f"rec{h}_{comp}")
                nc.scalar.activation(rec, lnt, func=EXP, scale=-1.0)
                recs.append(rec)
                psb = pPsb.tile([128, 1024], MDT, tag="psb", name=f"psb{h}_{comp}")
                nc.vector.tensor_copy(psb, pp)
                psbs.append(psb)

            return boundary

        def emit_comp(h, comp, qs, qw, ks, kn, v2t, recs, psbs, filler=None):
            ks_t = kn if comp == 0 else ks
            qs_t = qs if comp == 0 else qw
            sums = ps_sums.tile([128, 1024], F32, tag="sums", name=f"sums{h}_{comp}")
            pp = ps_p.tile([128, 1024], F32, tag="p", name=f"pp{h}_{comp}")
            p0 = pp[:, 0:512]
            p1 = pp[:, 512:1024]
            for tk in range(TC):
                st = ps_pair.tile([128, 1024], F32, tag="pair", name=f"st{h}_{comp}_{tk}")
                ksl = slice(tk * 128, (tk + 1) * 128)
                mm(st[:, 0:512], ks_t[:, ksl], qs_t[:, 0:512], True, True)
                mm(st[:, 512:1024], ks_t[:, ksl], qs_t[:, 512:1024], True, True)
                e = pE.tile([128, 1024], MDT, tag="e", name=f"e{h}_{comp}_{tk}")
                nc.scalar.activation(e, st, func=EXP)
                if tk == 0 and filler is not None:
                    filler()
                if tk == 0 and state["post"] is not None:
                    flush_post()
                else:
                    flush()
                if comp == 0:
                    vt = v1[:, tk * 1024 + h * 128 : tk * 1024 + h * 128 + 128]
                else:
                    vt = v2t[:, tk * 128 : (tk + 1) * 128]
                state["pending"] = (sums, p0, p1, e, vt, tk == 0, tk == TC - 1)
            state["post"] = make_boundary(h, comp, sums, pp, recs, psbs)

        def emit_norm(h, recs, psbs):
            osl = slice(h * 1024, (h + 1) * 1024)
            t1 = pEs.tile([128, 1024], MDT, tag="es", name=f"t1_{h}")
            nc.vector.tensor_mul(t1, psbs[0], recs[0])
            nc.vector.tensor_mul(osb[:, osl], psbs[1], recs[1])
            nc.vector.tensor_add(osb[:, osl], osb[:, osl], t1)

        # prologue: head 0 projections
        prefetch_w(0)
        for kk in range(KC):
            tk_ = pB.tile([128, S], MDT, tag="bigB", name=f"xtk{kk}")
            dma_split(tk_, d_xtk[kk], n=2)
            xtk.append(tk_)
        prefetch_w(1)
        for kk in range(KC):
            tq = pA.tile([128, S], MDT, tag="bigA", name=f"xtq{kk}")
            dma_split(tq, d_xtq[kk], n=2)
            xtq.append(tq)
        kp = emit_k_chain(0)
        ks0, kn0 = finish_k(0, kp)
        qp = emit_q_chain(0)
        qs0, qw0 = finish_q(0, qp)
        cur = (qs0, qw0, ks0, kn0, build_v2(0))

        kp_box = {}
        for h in range(H):
            prefetch_w(h + 2)
            recs, psbs = [], []
            emit_comp(h, 0, *cur, recs, psbs)
            # splice the next head's K chain into the comp0->comp1 boundary
            # (PE filler while exp(c0,7) + ln/rec complete)
            filler = None
            if h + 1 < H:
                def filler(hh=h + 1):
                    kp_box["kp"] = emit_k_chain(hh)
            emit_comp(h, 1, *cur, recs, psbs, filler=filler)
            if h + 1 < H:
                ksn, knn = finish_k(h + 1, kp_box.pop("kp"))
                qp = emit_q_chain(h + 1)  # PE filler for the c1-iter7 flush
                flush_post()  # c1 final AV+sums + boundary (rec_i, P2 evac)
                qsn, qwn = finish_q(h + 1, qp)
                nxt = (qsn, qwn, ksn, knn, build_v2(h + 1))
            else:
                # last head: prefetch wo during the tail
                wo_t = []
                for hh in range(H):
                    tw = pA.tile([128, 1024], MDT, tag="bigA", name=f"wo{hh}")
                    nc.sync.dma_start(out=tw, in_=d_wo[hh])
                    wo_t.append(tw)
                flush_post()
                nxt = None
            emit_norm(h, recs, psbs)
            cur = nxt

        # ---- output projection (transposed: out[odim, tok]) ----
        for oc in range(KC):
            pair = ps_pair.tile([128, 1024], F32, tag="pair", name=f"ops{oc}")
            osl = slice(oc * 128, (oc + 1) * 128)
            for hh in range(H):
                lhsT = wo_t[hh][:, osl]
                hb = hh * 1024
                mm(pair[:, 0:512], lhsT, osb[:, hb : hb + 512], hh == 0, hh == H - 1)
                mm(
                    pair[:, 512:1024],
                    lhsT,
                    osb[:, hb + 512 : hb + 1024],
                    hh == 0,
                    hh == H - 1,
                )
            oev = pB.tile([128, 1024], F32, tag="bigB", name=f"oev{oc}")
            if oc % 2 == 0:
                nc.scalar.copy(oev, pair)
            else:
                nc.vector.tensor_copy(oev, pair)
            nc.sync.dma_start(out=d_out[oc][:, 0:512], in_=oev[:, 0:512])
            nc.sync.dma_start(out=d_out[oc][:, 512:1024], in_=oev[:, 512:1024])

    _split_waits(nc)
    return nc


_NC_CACHE = {}


def kernel(
    queries,
    keys,
    values,
    wq_r,
    wq_i,
    wk_r,
    wk_i,
    wv_r,
    wv_i,
    wo_r,
    wo_i,
    _trace=False,
):
    global LAST_EXEC_NS
    _install_axon_profile_shim()
    _install_tile_drain_patch()
    from concourse.bass_utils import run_bass_kernel_spmd

    scale = 1.0 / np.sqrt(DH)
    WQ = _head_tiles(_build_wqk(np.asarray(wq_r), np.asarray(wq_i), scale))
    WK = _head_tiles(_build_wqk(np.asarray(wk_r), np.asarray(wk_i), 1.0))
    WV = _kchunk_tiles(_build_wqk(np.asarray(wv_r), np.asarray(wv_i), 1.0))
    WO = _kchunk_tiles(_build_wo(np.asarray(wo_r), np.asarray(wo_i)))
    CST = np.zeros((128, 320), np.float32)
    CST[:, 0:128] = 1.0

    queries = np.asarray(queries)
    keys = np.asarray(keys)
    values = np.asarray(values)

    in_maps = []
    for b in range(NCORES):
        in_maps.append(
            {
                "xtq": _xt(queries[b]).reshape(KC, 128, S),
                "xtk": _xt(keys[b]).reshape(KC, 128, S),
                "xtv": _xt(values[b]).reshape(KC, 128, S),
                "wq": WQ,
                "wk": WK,
                "wv": WV,
                "wo": WO,
                "cst": CST,
            }
        )

    if "nc" not in _NC_CACHE:
        _NC_CACHE["nc"] = _build_nc()
    nc = _NC_CACHE["nc"]

    res = run_bass_kernel_spmd(nc, in_maps, list(range(NCORES)), trace=_trace)
    LAST_EXEC_NS = res.exec_time_ns

    out = np.empty((B, S, D, 2), np.float32)
    for b in range(NCORES):
        # res: [oc, odim, tok] -> [tok, oc*128+odim] -> [S, D, 2]
        r = res.results[b]["out"].reshape(1024, S)
        out[b] = r.T.reshape(S, D, 2)
    return out


# revision 14
# speedup vs baseline: 1.1461x; 1.0462x over previous
"""Complex multi-head attention on 8 Trainium2 cores (Bass/Tile).

Sharding: pure data-parallel over batch (B=8 -> 1 batch per core),
weights replicated. No collectives.

v2 redesign (vs baseline at 465us):
  - Every LDWEIGHTS serves 2 matmuls (the two 512-col halves of each
    1024-wide rhs share the stationary operand) so the fp32 weight-load
    (224ns, no FWL for fp32) hides under 2x213ns of streaming.
  - Scores for both halves land in one 2-bank PSUM pair tile and are
    exponentiated with a single [128,1024] activation (amortizes the
    ~300-cycle ACT instruction overhead).
  - Softmax reciprocal moved off the DVE (was 32 x 3.9us of iterative
    divide) to the scalar engine as exp(-ln(sums)); Ln+Exp share one
    ACT table set.
  - P1/P2 evacuated unnormalized; normalization (3 DVE ops/head)
    happens off the critical path during the next head's projections.
  - Lag-1 software pipeline: st pair of iter i+1 is emitted between
    exp(i) and the sums/AV matmuls of iter i, so the PE never waits on
    the scalar engine; next head's K/Q projection chains are spliced
    into the head boundary to cover the last exp + rec latency.
  - O-projection computed transposed (out = [odim, tok]) with
    wo-stationary h-chains so its LDW is also paired; host undoes the
    transpose.
PSUM: st/proj/V/O pair pool [128,1024]x2 (4 banks) + sums pair
[128,1024]x1 (2) + P accumulators [128,512]x2 (2) = 8 banks exactly.
"""

import sys
import types
import numpy as np

B, S, D, H = 8, 1024, 512, 8
DH = D // H
KC = 8  # k-chunks of 128 over (c,d) = 1024
TC = 8  # token chunks of 128
NCORES = 8

LAST_EXEC_NS = None


# ---------------------------------------------------------------- shims
def _install_axon_profile_shim():
    if "antenv.axon_hooks" in sys.modules:
        return
    try:
        import antenv  # noqa: F401

        mod = types.ModuleType("antenv.axon_hooks")
        state = {"hook": None}
        mod.set_axon_ntff_profile_hook = lambda h: state.__setitem__("hook", h)
        mod.get_axon_ntff_profile_hook = lambda: state["hook"]
        sys.modules["antenv.axon_hooks"] = mod
        from trn_agent_boot.trn_boot import _ntff_profile_via_ctypes

        hook = _ntff_profile_via_ctypes("/opt/axon/libaxon_pjrt.so")
        if hook is not None:
            mod.set_axon_ntff_profile_hook(hook)
    except Exception:
        pass


def _install_tile_drain_patch():
    """This walrus build allows ONE sync wait per instruction; split the
    TileContext exit drain's waits across preceding sync NOPs."""
    import concourse.mybir as mybir
    import concourse.tile as tile
    from concourse.vector_clock import ScopedClock

    if getattr(tile.TileContext, "_drain_patched", False):
        return

    def _patched(self, tick_clock, wait_clock):
        probe = mybir.InstNoOp(name="I-drain-probe")
        probe.engine = mybir.EngineType.SP
        wait_clock.add_sem_waits(probe, ScopedClock({None: tick_clock.global_clock}))
        waits = list(probe.sync_info.on_wait or []) if probe.sync_info else []
        for w in waits:
            nop = self.nc.sync.nop()
            nop.ins.sync_info = mybir.SyncInfo(on_wait=[w], on_update=[])
        self.nc.sync.drain()
        self.nc.all_engine_barrier()
        assert self.sems is not None
        popped = self.nc._tile_sem_poison_stack.pop()
        assert popped is self._sem_poison
        self.nc.clear_and_free_semaphores(list(self.sems.allocated().values()))
        self.nc.all_engine_barrier()

    tile.TileContext._drain_and_barrier = _patched
    tile.TileContext._drain_patched = True


def _split_waits(nc, max_waits=1):
    """Hoist extra sync waits onto preceding same-engine NOPs (walrus here
    rejects >1 sync wait per instruction)."""
    import concourse.mybir as mybir

    def process(blk):
        lst = blk.instructions
        i = 0
        while i < len(lst):
            inst = lst[i]
            if hasattr(inst, "blocks"):
                for b in inst.blocks or []:
                    process(b)
            si = inst.sync_info
            if si is not None and si.on_wait and len(si.on_wait) > max_waits:
                waits = list(si.on_wait)
                keep, extra = waits[-max_waits:], waits[:-max_waits]
                inst.sync_info = mybir.SyncInfo(
                    on_wait=keep, on_update=list(si.on_update or [])
                )
                for j, w in enumerate(extra):
                    nop = mybir.InstNoOp(name=f"{inst.name}-ws{j}")
                    nop.engine = inst.engine
                    nop.sync_info = mybir.SyncInfo(on_wait=[w], on_update=[])
                    lst.insert(i, nop)
                    i += 1
            i += 1

    for f in nc.m.functions:
        for blk in f.blocks:
            process(blk)


# ------------------------------------------------------------ host prep
def _build_wqk(wr, wi, scale):
    """[1024 k=(c,d), 1024 m=(h, c', dh)] for Q/K projections."""
    W = np.empty((2 * D, 2 * D), np.float32)
    for h in range(H):
        o = slice(h * DH, (h + 1) * DH)
        c0 = h * 2 * DH
        W[0:D, c0 : c0 + DH] = wr[o].T * scale
        W[D:, c0 : c0 + DH] = -wi[o].T * scale
        W[0:D, c0 + DH : c0 + 2 * DH] = wi[o].T * scale
        W[D:, c0 + DH : c0 + 2 * DH] = wr[o].T * scale
    return W


def _head_tiles(W):
    """[1024,1024] -> [H, 128, 1024]: per-head column block, k-chunk cols."""
    out = np.empty((H, 128, 1024), np.float32)
    for h in range(H):
        blk = W[:, h * 128 : (h + 1) * 128]  # [1024, 128]
        for kk in range(KC):
            out[h, :, kk * 128 : (kk + 1) * 128] = blk[kk * 128 : (kk + 1) * 128]
    return out


def _kchunk_tiles(W):
    """[1024,1024] -> [KC, 128, 1024]: row chunks."""
    return np.ascontiguousarray(W.reshape(KC, 128, 1024))


def _build_wo(wo_r, wo_i):
    """rows (h, c', dh), cols (o, c) interleaved to match [S, D, 2]."""
    W = np.empty((2 * D, 2 * D), np.float32)
    for h in range(H):
        d = slice(h * DH, (h + 1) * DH)
        r0 = h * 2 * DH
        W[r0 : r0 + DH, 0::2] = wo_r[:, d].T
        W[r0 : r0 + DH, 1::2] = wo_i[:, d].T
        W[r0 + DH : r0 + 2 * DH, 0::2] = -wo_i[:, d].T
        W[r0 + DH : r0 + 2 * DH, 1::2] = wo_r[:, d].T
    return W


def _xt(x):  # [S, D, 2] -> [2D, S] feature-major
    out = np.empty((2 * D, S), np.float32)
    out[0:D] = x[:, :, 0].T
    out[D:] = x[:, :, 1].T
    return out


# ------------------------------------------------------------ bass build
def _build_nc():
    import concourse.bass as bass
    import concourse.mybir as mybir
    import concourse.tile as tile
    from contextlib import ExitStack

    MDT = mybir.dt.float32r
    F32 = mybir.dt.float32
    BF16 = mybir.dt.bfloat16
    EXP = mybir.ActivationFunctionType.Exp
    LN = mybir.ActivationFunctionType.Ln

    nc = bass.Bass()
    d_xtq = nc.dram_tensor("xtq", [KC, 128, S], MDT, kind="ExternalInput")
    d_xtk = nc.dram_tensor("xtk", [KC, 128, S], MDT, kind="ExternalInput")
    d_xtv = nc.dram_tensor("xtv", [KC, 128, S], MDT, kind="ExternalInput")
    d_wq = nc.dram_tensor("wq", [H, 128, 1024], MDT, kind="ExternalInput")
    d_wk = nc.dram_tensor("wk", [H, 128, 1024], MDT, kind="ExternalInput")
    d_wv = nc.dram_tensor("wv", [KC, 128, 1024], MDT, kind="ExternalInput")
    d_wo = nc.dram_tensor("wo", [H, 128, 1024], MDT, kind="ExternalInput")
    d_cst = nc.dram_tensor("cst", [128, 320], MDT, kind="ExternalInput")
    d_cstb = nc.dram_tensor("cstb", [128, 128], BF16, kind="ExternalInput")
    # transposed output: [oc, odim, tok]
    d_out = nc.dram_tensor("out", [KC, 128, S], F32, kind="ExternalOutput")

    with tile.TileContext(nc) as tc, ExitStack() as ctx:
        ctx.enter_context(
            nc.allow_low_precision(reason="float32r tiles are bit-identical fp32")
        )
        pA = ctx.enter_context(tc.tile_pool(name="bigA", bufs=8))  # xtv->xtq->wo
        pB = ctx.enter_context(tc.tile_pool(name="bigB", bufs=8))  # wv->xtk->oev
        pV1 = ctx.enter_context(tc.tile_pool(name="v1", bufs=1))
        pOsb = ctx.enter_context(tc.tile_pool(name="osb", bufs=1))
        pQs = ctx.enter_context(tc.tile_pool(name="qs", bufs=2))
        pKs = ctx.enter_context(tc.tile_pool(name="ks", bufs=2))
        pQw = ctx.enter_context(tc.tile_pool(name="qw", bufs=1))
        pKn = ctx.enter_context(tc.tile_pool(name="kn", bufs=1))
        pV2 = ctx.enter_context(tc.tile_pool(name="v2", bufs=1))
        pE = ctx.enter_context(tc.tile_pool(name="e", bufs=4))
        pEs = ctx.enter_context(tc.tile_pool(name="es", bufs=4))
        pPsb = ctx.enter_context(tc.tile_pool(name="psb", bufs=2))
        pT = ctx.enter_context(tc.tile_pool(name="t", bufs=1))
        pRec = ctx.enter_context(tc.tile_pool(name="rec", bufs=3))
        pWqk = ctx.enter_context(tc.tile_pool(name="wqk", bufs=3))
        pC = ctx.enter_context(tc.tile_pool(name="const", bufs=1))

        ps_pair = ctx.enter_context(tc.tile_pool(name="ps_pair", bufs=2, space="PSUM"))
        ps_sums = ctx.enter_context(tc.tile_pool(name="ps_sums", bufs=1, space="PSUM"))
        ps_p = ctx.enter_context(tc.tile_pool(name="ps_p", bufs=1, space="PSUM"))

        cst = pC.tile([128, 320], MDT, tag="cst")
        nc.sync.dma_start(out=cst, in_=d_cst[:, :])
        onesb = pC.tile([128, 128], BF16, tag="cstb")
        nc.sync.dma_start(out=onesb, in_=d_cstb[:, :])

        def mm(out, lhsT, rhs, start, stop):
            nc.tensor.matmul(out, lhsT=lhsT, rhs=rhs, start=start, stop=stop)

        # ---- phase V: V projection (all heads) ----
        def dma_split(out, in_, n=4):
            w = out.shape[-1] // n
            for i in range(n):
                nc.sync.dma_start(
                    out=out[:, i * w : (i + 1) * w], in_=in_[:, i * w : (i + 1) * w]
                )

        xtv, wv = [], []
        for kk in range(KC):
            tv = pA.tile([128, S], MDT, tag="bigA", name=f"xtv{kk}")
            nc.sync.dma_start(out=tv, in_=d_xtv[kk])
            xtv.append(tv)
            tw = pB.tile([128, 1024], MDT, tag="bigB", name=f"wv{kk}")
            nc.sync.dma_start(out=tw, in_=d_wv[kk])
            wv.append(tw)

        v1 = pV1.tile([128, TC * 1024], BF16, tag="v1")  # [p, (t_, h, dh2)]
        for t_ in range(TC):
            pair = ps_pair.tile([128, 1024], F32, tag="pair", name=f"vps{t_}")
            tsl = slice(t_ * 128, (t_ + 1) * 128)
            for kk in range(KC):
                lhsT = xtv[kk][:, tsl]
                mm(pair[:, 0:512], lhsT, wv[kk][:, 0:512], kk == 0, kk == KC - 1)
                mm(pair[:, 512:1024], lhsT, wv[kk][:, 512:1024], kk == 0, kk == KC - 1)
            nc.vector.tensor_copy(v1[:, t_ * 1024 : (t_ + 1) * 1024], pair)

        # ---- load XT_q / XT_k (reuse pA / pB slots) ----
        xtq, xtk = [], []

        osb = pOsb.tile([128, H * 1024], MDT, tag="osb")  # [p, (h, tok)]

        # ---------------- per-head helpers ----------------
        wq_t, wk_t = {}, {}

        def prefetch_w(h):
            if h >= H or h in wq_t:
                return
            tk2 = pWqk.tile([128, 1024], MDT, tag="wqk", name=f"wk{h}")
            dma_split(tk2, d_wk[h], n=2)
            wk_t[h] = tk2
            tq = pWqk.tile([128, 1024], MDT, tag="wqk", name=f"wq{h}")
            dma_split(tq, d_wq[h], n=2)
            wq_t[h] = tq

        def proj_chain(w_tile, xt_tiles, name):
            pair = ps_pair.tile([128, 1024], F32, tag="pair", name=name)
            for kk in range(KC):
                lhsT = w_tile[:, kk * 128 : (kk + 1) * 128]
                mm(pair[:, 0:512], lhsT, xt_tiles[kk][:, 0:512], kk == 0, kk == KC - 1)
                mm(
                    pair[:, 512:1024],
                    lhsT,
                    xt_tiles[kk][:, 512:1024],
                    kk == 0,
                    kk == KC - 1,
                )
            return pair

        def emit_k_chain(h):
            return proj_chain(wk_t.pop(h), xtk, f"kproj{h}")

        def finish_k(h, pair):
            ks = pKs.tile([128, S], MDT, tag="ks", name=f"kstack{h}")
            nc.vector.tensor_copy(ks, pair)
            kn = pKn.tile([128, S], MDT, tag="kn", name=f"kneg{h}")
            nc.vector.tensor_copy(kn[0:64, :], ks[0:64, :])
            nc.vector.tensor_scalar_mul(kn[64:128, :], ks[64:128, :], -1.0)
            return ks, kn

        def emit_q_chain(h):
            return proj_chain(wq_t.pop(h), xtq, f"qproj{h}")

        def finish_q(h, pair):
            qs = pQs.tile([128, S], MDT, tag="qs", name=f"qstack{h}")
            nc.vector.tensor_copy(qs[:, 0:512], pair[:, 0:512])
            nc.vector.tensor_copy(qs[:, 512:1024], pair[:, 512:1024])
            qw = pQw.tile([128, S], MDT, tag="qw", name=f"qswap{h}")
            nc.sync.dma_start(out=qw[0:64, :], in_=qs[64:128, :])
            nc.sync.dma_start(out=qw[64:128, :], in_=qs[0:64, :])
            return qs, qw

        def build_v2(h):
            v2t = pV2.tile([128, 1024], BF16, tag="v2", name=f"v2h{h}")
            v1v = v1.rearrange("p (t h d) -> p t h d", t=TC, h=H, d=128)
            v2v = v2t.rearrange("p (t d) -> p t d", t=TC, d=128)
            nc.vector.tensor_scalar_mul(
                v2v[:, :, 0:64], v1v[:, :, h, 64:128], -1.0
            )
            nc.vector.tensor_copy(v2v[:, :, 64:128], v1v[:, :, h, 0:64])
            return v2t

        # ---------------- the pipelined head loop ----------------
        # pending = (sums, p0, p1, e, vt, first, last, boundary_cb)
        state = {"pending": None, "post": None}

        def flush_post():
            flush()
            sums, efin, boundary = state["post"]
            mm(sums[:, 0:512], onesb, efin[:, 0:512], True, True)
            mm(sums[:, 512:1024], onesb, efin[:, 512:1024], True, True)
            boundary()
            state["post"] = None

        def flush():
            p = state["pending"]
            if p is None:
                return
            p0, p1, e, vt, first, last = p
            mm(p0, vt, e[:, 0:512], first, last)
            mm(p1, vt, e[:, 512:1024], first, last)
            state["pending"] = None

        def make_boundary(h, comp, sums, pp, recs, psbs):
            def boundary():
                lnt = pRec.tile([128, 1024], MDT, tag="rec", name=f"lnt{h}_{comp}")
                nc.scalar.activation(lnt, sums, func=LN)
                rec = pRec.tile([128, 1024], MDT, tag="rec", name=f"rec{h}_{comp}")
                nc.scalar.activation(rec, lnt, func=EXP, scale=-1.0)
                recs.append(rec)
                psb = pPsb.tile([128, 1024], MDT, tag="psb", name=f"psb{h}_{comp}")
                nc.vector.tensor_copy(psb, pp)
                psbs.append(psb)

            return boundary

        def emit_comp(h, comp, qs, qw, ks, kn, v2t, recs, psbs, filler=None):
            ks_t = kn if comp == 0 else ks
            qs_t = qs if comp == 0 else qw
            sums = ps_sums.tile([128, 1024], F32, tag="sums", name=f"sums{h}_{comp}")
            pp = ps_p.tile([128, 1024], F32, tag="p", name=f"pp{h}_{comp}")
            p0 = pp[:, 0:512]
            p1 = pp[:, 512:1024]
            es_lvl = {1: [], 2: []}
            e_prev = None
            for tk in range(TC):
                st = ps_pair.tile([128, 1024], F32, tag="pair", name=f"st{h}_{comp}_{tk}")
                ksl = slice(tk * 128, (tk + 1) * 128)
                mm(st[:, 0:512], ks_t[:, ksl], qs_t[:, 0:512], True, True)
                mm(st[:, 512:1024], ks_t[:, ksl], qs_t[:, 512:1024], True, True)
                e = pE.tile([128, 1024], BF16, tag="e", name=f"e{h}_{comp}_{tk}")
                nc.scalar.activation(e, st, func=EXP)
                # bf16 pairwise sum tree on the DVE (replaces 14 of the 16
                # ones-matmuls per comp)
                if tk % 2 == 1:
                    es = pEs.tile([128, 1024], BF16, tag="es", name=f"es{h}_{comp}_{tk}")
                    nc.vector.tensor_add(es, e_prev, e)
                    es_lvl[1].append(es)
                    if len(es_lvl[1]) == 2:
                        a, b = es_lvl[1]
                        es_lvl[1] = []
                        es2 = pEs.tile(
                            [128, 1024], BF16, tag="es", name=f"es2{h}_{comp}_{tk}"
                        )
                        nc.vector.tensor_add(es2, a, b)
                        es_lvl[2].append(es2)
                if tk == 0 and filler is not None:
                    filler()
                if tk == 0 and state["post"] is not None:
                    flush_post()
                else:
                    flush()
                if comp == 0:
                    vt = v1[:, tk * 1024 + h * 128 : tk * 1024 + h * 128 + 128]
                else:
                    vt = v2t[:, tk * 128 : (tk + 1) * 128]
                state["pending"] = (p0, p1, e, vt, tk == 0, tk == TC - 1)
                e_prev = e
            a, b = es_lvl[2]
            efin = pEs.tile([128, 1024], BF16, tag="es", name=f"ef{h}_{comp}")
            nc.vector.tensor_add(efin, a, b)
            state["post"] = (sums, efin, make_boundary(h, comp, sums, pp, recs, psbs))

        def emit_norm(h, recs, psbs):
            osl = slice(h * 1024, (h + 1) * 1024)
            t1 = pT.tile([128, 1024], MDT, tag="t", name=f"t1_{h}")
            nc.vector.tensor_mul(t1, psbs[0], recs[0])
            nc.vector.tensor_mul(osb[:, osl], psbs[1], recs[1])
            nc.vector.tensor_add(osb[:, osl], osb[:, osl], t1)

        # prologue: head 0 projections
        prefetch_w(0)
        for kk in range(KC):
            tk_ = pB.tile([128, S], MDT, tag="bigB", name=f"xtk{kk}")
            dma_split(tk_, d_xtk[kk], n=2)
            xtk.append(tk_)
        prefetch_w(1)
        for kk in range(KC):
            tq = pA.tile([128, S], MDT, tag="bigA", name=f"xtq{kk}")
            dma_split(tq, d_xtq[kk], n=2)
            xtq.append(tq)
        kp = emit_k_chain(0)
        ks0, kn0 = finish_k(0, kp)
        qp = emit_q_chain(0)
        qs0, qw0 = finish_q(0, qp)
        cur = (qs0, qw0, ks0, kn0, build_v2(0))

        kp_box = {}
        for h in range(H):
            prefetch_w(h + 2)
            recs, psbs = [], []
            emit_comp(h, 0, *cur, recs, psbs)
            # splice the next head's K chain into the comp0->comp1 boundary
            # (PE filler while exp(c0,7) + ln/rec complete)
            filler = None
            if h + 1 < H:
                def filler(hh=h + 1):
                    kp_box["kp"] = emit_k_chain(hh)
            emit_comp(h, 1, *cur, recs, psbs, filler=filler)
            if h + 1 < H:
                ksn, knn = finish_k(h + 1, kp_box.pop("kp"))
                qp = emit_q_chain(h + 1)  # PE filler for the c1-iter7 flush
                flush_post()  # c1 final AV+sums + boundary (rec_i, P2 evac)
                qsn, qwn = finish_q(h + 1, qp)
                nxt = (qsn, qwn, ksn, knn, build_v2(h + 1))
            else:
                # last head: prefetch wo during the tail
                wo_t = []
                for hh in range(H):
                    tw = pA.tile([128, 1024], MDT, tag="bigA", name=f"wo{hh}")
                    nc.sync.dma_start(out=tw, in_=d_wo[hh])
                    wo_t.append(tw)
                flush_post()
                nxt = None
            emit_norm(h, recs, psbs)
            cur = nxt

        # ---- output projection (transposed: out[odim, tok]) ----
        for oc in range(KC):
            pair = ps_pair.tile([128, 1024], F32, tag="pair", name=f"ops{oc}")
            osl = slice(oc * 128, (oc + 1) * 128)
            for hh in range(H):
                lhsT = wo_t[hh][:, osl]
                hb = hh * 1024
                mm(pair[:, 0:512], lhsT, osb[:, hb : hb + 512], hh == 0, hh == H - 1)
                mm(
                    pair[:, 512:1024],
                    lhsT,
                    osb[:, hb + 512 : hb + 1024],
                    hh == 0,
                    hh == H - 1,
                )
            oev = pB.tile([128, 1024], F32, tag="bigB", name=f"oev{oc}")
            if oc % 2 == 0:
                nc.scalar.copy(oev, pair)
            else:
                nc.vector.tensor_copy(oev, pair)
            nc.sync.dma_start(out=d_out[oc][:, 0:512], in_=oev[:, 0:512])
            nc.sync.dma_start(out=d_out[oc][:, 512:1024], in_=oev[:, 512:1024])

    _split_waits(nc)
    return nc


_NC_CACHE = {}


def kernel(
    queries,
    keys,
    values,
    wq_r,
    wq_i,
    wk_r,
    wk_i,
    wv_r,
    wv_i,
    wo_r,
    wo_i,
    _trace=False,
):
    global LAST_EXEC_NS
    _install_axon_profile_shim()
    _install_tile_drain_patch()
    from concourse.bass_utils import run_bass_kernel_spmd

    scale = 1.0 / np.sqrt(DH)
    WQ = _head_tiles(_build_wqk(np.asarray(wq_r), np.asarray(wq_i), scale))
    WK = _head_tiles(_build_wqk(np.asarray(wk_r), np.asarray(wk_i), 1.0))
    WV = _kchunk_tiles(_build_wqk(np.asarray(wv_r), np.asarray(wv_i), 1.0))
    WO = _kchunk_tiles(_build_wo(np.asarray(wo_r), np.asarray(wo_i)))
    CST = np.zeros((128, 320), np.float32)
    CST[:, 0:128] = 1.0
    import ml_dtypes
    CSTB = np.ones((128, 128), ml_dtypes.bfloat16)

    queries = np.asarray(queries)
    keys = np.asarray(keys)
    values = np.asarray(values)

    in_maps = []
    for b in range(NCORES):
        in_maps.append(
            {
                "xtq": _xt(queries[b]).reshape(KC, 128, S),
                "xtk": _xt(keys[b]).reshape(KC, 128, S),
                "xtv": _xt(values[b]).reshape(KC, 128, S),
                "wq": WQ,
                "wk": WK,
                "wv": WV,
                "wo": WO,
                "cst": CST,
                "cstb": CSTB,
            }
        )

    if "nc" not in _NC_CACHE:
        _NC_CACHE["nc"] = _build_nc()
    nc = _NC_CACHE["nc"]

    res = run_bass_kernel_spmd(nc, in_maps, list(range(NCORES)), trace=_trace)
    LAST_EXEC_NS = res.exec_time_ns

    out = np.empty((B, S, D, 2), np.float32)
    for b in range(NCORES):
        # res: [oc, odim, tok] -> [tok, oc*128+odim] -> [S, D, 2]
        r = res.results[b]["out"].reshape(1024, S)
        out[b] = r.T.reshape(S, D, 2)
    return out


# revision 15
# speedup vs baseline: 1.2164x; 1.0614x over previous
"""Complex multi-head attention on 8 Trainium2 cores (Bass/Tile).

Sharding: pure data-parallel over batch (B=8 -> 1 batch per core),
weights replicated. No collectives.

v2 redesign (vs baseline at 465us):
  - Every LDWEIGHTS serves 2 matmuls (the two 512-col halves of each
    1024-wide rhs share the stationary operand) so the fp32 weight-load
    (224ns, no FWL for fp32) hides under 2x213ns of streaming.
  - Scores for both halves land in one 2-bank PSUM pair tile and are
    exponentiated with a single [128,1024] activation (amortizes the
    ~300-cycle ACT instruction overhead).
  - Softmax reciprocal moved off the DVE (was 32 x 3.9us of iterative
    divide) to the scalar engine as exp(-ln(sums)); Ln+Exp share one
    ACT table set.
  - P1/P2 evacuated unnormalized; normalization (3 DVE ops/head)
    happens off the critical path during the next head's projections.
  - Lag-1 software pipeline: st pair of iter i+1 is emitted between
    exp(i) and the sums/AV matmuls of iter i, so the PE never waits on
    the scalar engine; next head's K/Q projection chains are spliced
    into the head boundary to cover the last exp + rec latency.
  - O-projection computed transposed (out = [odim, tok]) with
    wo-stationary h-chains so its LDW is also paired; host undoes the
    transpose.
PSUM: st/proj/V/O pair pool [128,1024]x2 (4 banks) + sums pair
[128,1024]x1 (2) + P accumulators [128,512]x2 (2) = 8 banks exactly.
"""

import sys
import types
import numpy as np

B, S, D, H = 8, 1024, 512, 8
DH = D // H
KC = 8  # k-chunks of 128 over (c,d) = 1024
TC = 8  # token chunks of 128
NCORES = 8

LAST_EXEC_NS = None


# ---------------------------------------------------------------- shims
def _install_axon_profile_shim():
    if "antenv.axon_hooks" in sys.modules:
        return
    try:
        import antenv  # noqa: F401

        mod = types.ModuleType("antenv.axon_hooks")
        state = {"hook": None}
        mod.set_axon_ntff_profile_hook = lambda h: state.__setitem__("hook", h)
        mod.get_axon_ntff_profile_hook = lambda: state["hook"]
        sys.modules["antenv.axon_hooks"] = mod
        from trn_agent_boot.trn_boot import _ntff_profile_via_ctypes

        hook = _ntff_profile_via_ctypes("/opt/axon/libaxon_pjrt.so")
        if hook is not None:
            mod.set_axon_ntff_profile_hook(hook)
    except Exception:
        pass


def _install_tile_drain_patch():
    """This walrus build allows ONE sync wait per instruction; split the
    TileContext exit drain's waits across preceding sync NOPs."""
    import concourse.mybir as mybir
    import concourse.tile as tile
    from concourse.vector_clock import ScopedClock

    if getattr(tile.TileContext, "_drain_patched", False):
        return

    def _patched(self, tick_clock, wait_clock):
        probe = mybir.InstNoOp(name="I-drain-probe")
        probe.engine = mybir.EngineType.SP
        wait_clock.add_sem_waits(probe, ScopedClock({None: tick_clock.global_clock}))
        waits = list(probe.sync_info.on_wait or []) if probe.sync_info else []
        for w in waits:
            nop = self.nc.sync.nop()
            nop.ins.sync_info = mybir.SyncInfo(on_wait=[w], on_update=[])
        self.nc.sync.drain()
        self.nc.all_engine_barrier()
        assert self.sems is not None
        popped = self.nc._tile_sem_poison_stack.pop()
        assert popped is self._sem_poison
        self.nc.clear_and_free_semaphores(list(self.sems.allocated().values()))
        self.nc.all_engine_barrier()

    tile.TileContext._drain_and_barrier = _patched
    tile.TileContext._drain_patched = True


def _split_waits(nc, max_waits=1):
    """Hoist extra sync waits onto preceding same-engine NOPs (walrus here
    rejects >1 sync wait per instruction)."""
    import concourse.mybir as mybir

    def process(blk):
        lst = blk.instructions
        i = 0
        while i < len(lst):
            inst = lst[i]
            if hasattr(inst, "blocks"):
                for b in inst.blocks or []:
                    process(b)
            si = inst.sync_info
            if si is not None and si.on_wait and len(si.on_wait) > max_waits:
                waits = list(si.on_wait)
                keep, extra = waits[-max_waits:], waits[:-max_waits]
                inst.sync_info = mybir.SyncInfo(
                    on_wait=keep, on_update=list(si.on_update or [])
                )
                for j, w in enumerate(extra):
                    nop = mybir.InstNoOp(name=f"{inst.name}-ws{j}")
                    nop.engine = inst.engine
                    nop.sync_info = mybir.SyncInfo(on_wait=[w], on_update=[])
                    lst.insert(i, nop)
                    i += 1
            i += 1

    for f in nc.m.functions:
        for blk in f.blocks:
            process(blk)


# ------------------------------------------------------------ host prep
def _build_wqk(wr, wi, scale):
    """[1024 k=(c,d), 1024 m=(h, c', dh)] for Q/K projections."""
    W = np.empty((2 * D, 2 * D), np.float32)
    for h in range(H):
        o = slice(h * DH, (h + 1) * DH)
        c0 = h * 2 * DH
        W[0:D, c0 : c0 + DH] = wr[o].T * scale
        W[D:, c0 : c0 + DH] = -wi[o].T * scale
        W[0:D, c0 + DH : c0 + 2 * DH] = wi[o].T * scale
        W[D:, c0 + DH : c0 + 2 * DH] = wr[o].T * scale
    return W


def _head_tiles(W):
    """[1024,1024] -> [H, 128, 1024]: per-head column block, k-chunk cols."""
    out = np.empty((H, 128, 1024), np.float32)
    for h in range(H):
        blk = W[:, h * 128 : (h + 1) * 128]  # [1024, 128]
        for kk in range(KC):
            out[h, :, kk * 128 : (kk + 1) * 128] = blk[kk * 128 : (kk + 1) * 128]
    return out


def _kchunk_tiles(W):
    """[1024,1024] -> [KC, 128, 1024]: row chunks."""
    return np.ascontiguousarray(W.reshape(KC, 128, 1024))


def _build_wo(wo_r, wo_i):
    """rows (h, c', dh), cols (o, c) interleaved to match [S, D, 2]."""
    W = np.empty((2 * D, 2 * D), np.float32)
    for h in range(H):
        d = slice(h * DH, (h + 1) * DH)
        r0 = h * 2 * DH
        W[r0 : r0 + DH, 0::2] = wo_r[:, d].T
        W[r0 : r0 + DH, 1::2] = wo_i[:, d].T
        W[r0 + DH : r0 + 2 * DH, 0::2] = -wo_i[:, d].T
        W[r0 + DH : r0 + 2 * DH, 1::2] = wo_r[:, d].T
    return W


def _xt(x):  # [S, D, 2] -> [2D, S] feature-major
    out = np.empty((2 * D, S), np.float32)
    out[0:D] = x[:, :, 0].T
    out[D:] = x[:, :, 1].T
    return out


# ------------------------------------------------------------ bass build
def _build_nc():
    import concourse.bass as bass
    import concourse.mybir as mybir
    import concourse.tile as tile
    from contextlib import ExitStack

    MDT = mybir.dt.float32r
    F32 = mybir.dt.float32
    BF16 = mybir.dt.bfloat16
    EXP = mybir.ActivationFunctionType.Exp
    LN = mybir.ActivationFunctionType.Ln

    nc = bass.Bass()
    d_xtq = nc.dram_tensor("xtq", [KC, 128, S], MDT, kind="ExternalInput")
    d_xtk = nc.dram_tensor("xtk", [KC, 128, S], MDT, kind="ExternalInput")
    d_xtv = nc.dram_tensor("xtv", [KC, 128, S], BF16, kind="ExternalInput")
    d_wq = nc.dram_tensor("wq", [H, 128, 1024], MDT, kind="ExternalInput")
    d_wk = nc.dram_tensor("wk", [H, 128, 1024], MDT, kind="ExternalInput")
    d_wv = nc.dram_tensor("wv", [KC, 128, 1024], BF16, kind="ExternalInput")
    d_wo = nc.dram_tensor("wo", [H, 128, 1024], BF16, kind="ExternalInput")
    d_cst = nc.dram_tensor("cst", [128, 320], MDT, kind="ExternalInput")
    d_cstb = nc.dram_tensor("cstb", [128, 128], BF16, kind="ExternalInput")
    # transposed output: [oc, odim, tok]
    d_out = nc.dram_tensor("out", [KC, 128, S], F32, kind="ExternalOutput")

    with tile.TileContext(nc) as tc, ExitStack() as ctx:
        ctx.enter_context(
            nc.allow_low_precision(reason="float32r tiles are bit-identical fp32")
        )
        pA = ctx.enter_context(tc.tile_pool(name="bigA", bufs=8))  # xtv->xtq->wo
        pB = ctx.enter_context(tc.tile_pool(name="bigB", bufs=8))  # wv->xtk->oev
        pV1 = ctx.enter_context(tc.tile_pool(name="v1", bufs=1))
        pOsb = ctx.enter_context(tc.tile_pool(name="osb", bufs=1))
        pQs = ctx.enter_context(tc.tile_pool(name="qs", bufs=2))
        pKs = ctx.enter_context(tc.tile_pool(name="ks", bufs=2))
        pQw = ctx.enter_context(tc.tile_pool(name="qw", bufs=1))
        pKn = ctx.enter_context(tc.tile_pool(name="kn", bufs=1))
        pV2 = ctx.enter_context(tc.tile_pool(name="v2", bufs=1))
        pE = ctx.enter_context(tc.tile_pool(name="e", bufs=4))
        pEs = ctx.enter_context(tc.tile_pool(name="es", bufs=4))
        pPsb = ctx.enter_context(tc.tile_pool(name="psb", bufs=2))
        pRec = ctx.enter_context(tc.tile_pool(name="rec", bufs=3))
        pWqk = ctx.enter_context(tc.tile_pool(name="wqk", bufs=3))
        pC = ctx.enter_context(tc.tile_pool(name="const", bufs=1))

        ps_pair = ctx.enter_context(tc.tile_pool(name="ps_pair", bufs=2, space="PSUM"))
        ps_sums = ctx.enter_context(tc.tile_pool(name="ps_sums", bufs=1, space="PSUM"))
        ps_p = ctx.enter_context(tc.tile_pool(name="ps_p", bufs=1, space="PSUM"))

        cst = pC.tile([128, 320], MDT, tag="cst")
        nc.sync.dma_start(out=cst, in_=d_cst[:, :])
        onesb = pC.tile([128, 128], BF16, tag="cstb")
        nc.sync.dma_start(out=onesb, in_=d_cstb[:, :])

        def mm(out, lhsT, rhs, start, stop):
            nc.tensor.matmul(out, lhsT=lhsT, rhs=rhs, start=start, stop=stop)

        # ---- phase V: V projection (all heads) ----
        def dma_split(out, in_, n=4):
            w = out.shape[-1] // n
            for i in range(n):
                nc.sync.dma_start(
                    out=out[:, i * w : (i + 1) * w], in_=in_[:, i * w : (i + 1) * w]
                )

        xtv, wv = [], []
        for kk in range(KC):
            tv = pA.tile([128, S], BF16, tag="bigA", name=f"xtv{kk}")
            nc.sync.dma_start(out=tv, in_=d_xtv[kk])
            xtv.append(tv)
            tw = pB.tile([128, 1024], BF16, tag="bigB", name=f"wv{kk}")
            nc.sync.dma_start(out=tw, in_=d_wv[kk])
            wv.append(tw)

        v1 = pV1.tile([128, TC * 1024], BF16, tag="v1")  # [p, (t_, h, dh2)]
        for t_ in range(TC):
            pair = ps_pair.tile([128, 1024], F32, tag="pair", name=f"vps{t_}")
            tsl = slice(t_ * 128, (t_ + 1) * 128)
            for kk in range(KC):
                lhsT = xtv[kk][:, tsl]
                mm(pair[:, 0:512], lhsT, wv[kk][:, 0:512], kk == 0, kk == KC - 1)
                mm(pair[:, 512:1024], lhsT, wv[kk][:, 512:1024], kk == 0, kk == KC - 1)
            nc.vector.tensor_copy(v1[:, t_ * 1024 : (t_ + 1) * 1024], pair)

        # ---- load XT_q / XT_k (reuse pA / pB slots) ----
        xtq, xtk = [], []

        osb = pOsb.tile([128, H * 1024], BF16, tag="osb")  # [p, (h, tok)]

        # ---------------- per-head helpers ----------------
        wq_t, wk_t = {}, {}

        def prefetch_w(h):
            if h >= H or h in wq_t:
                return
            tk2 = pWqk.tile([128, 1024], MDT, tag="wqk", name=f"wk{h}")
            dma_split(tk2, d_wk[h], n=2)
            wk_t[h] = tk2
            tq = pWqk.tile([128, 1024], MDT, tag="wqk", name=f"wq{h}")
            dma_split(tq, d_wq[h], n=2)
            wq_t[h] = tq

        def proj_chain(w_tile, xt_tiles, name):
            pair = ps_pair.tile([128, 1024], F32, tag="pair", name=name)
            for kk in range(KC):
                lhsT = w_tile[:, kk * 128 : (kk + 1) * 128]
                mm(pair[:, 0:512], lhsT, xt_tiles[kk][:, 0:512], kk == 0, kk == KC - 1)
                mm(
                    pair[:, 512:1024],
                    lhsT,
                    xt_tiles[kk][:, 512:1024],
                    kk == 0,
                    kk == KC - 1,
                )
            return pair

        def emit_k_chain(h):
            return proj_chain(wk_t.pop(h), xtk, f"kproj{h}")

        def finish_k(h, pair):
            ks = pKs.tile([128, S], MDT, tag="ks", name=f"kstack{h}")
            nc.vector.tensor_copy(ks, pair)
            kn = pKn.tile([128, S], MDT, tag="kn", name=f"kneg{h}")
            nc.vector.tensor_copy(kn[0:64, :], ks[0:64, :])
            nc.vector.tensor_scalar_mul(kn[64:128, :], ks[64:128, :], -1.0)
            return ks, kn

        def emit_q_chain(h):
            return proj_chain(wq_t.pop(h), xtq, f"qproj{h}")

        def finish_q(h, pair):
            qs = pQs.tile([128, S], MDT, tag="qs", name=f"qstack{h}")
            nc.vector.tensor_copy(qs[:, 0:512], pair[:, 0:512])
            nc.vector.tensor_copy(qs[:, 512:1024], pair[:, 512:1024])
            qw = pQw.tile([128, S], MDT, tag="qw", name=f"qswap{h}")
            nc.sync.dma_start(out=qw[0:64, :], in_=qs[64:128, :])
            nc.sync.dma_start(out=qw[64:128, :], in_=qs[0:64, :])
            return qs, qw

        def build_v2(h):
            v2t = pV2.tile([128, 1024], BF16, tag="v2", name=f"v2h{h}")
            v1v = v1.rearrange("p (t h d) -> p t h d", t=TC, h=H, d=128)
            v2v = v2t.rearrange("p (t d) -> p t d", t=TC, d=128)
            nc.vector.tensor_scalar_mul(
                v2v[:, :, 0:64], v1v[:, :, h, 64:128], -1.0
            )
            nc.vector.tensor_copy(v2v[:, :, 64:128], v1v[:, :, h, 0:64])
            return v2t

        # ---------------- the pipelined head loop ----------------
        # pending = (sums, p0, p1, e, vt, first, last, boundary_cb)
        state = {"pending": None, "post": None}

        def flush_post():
            flush()
            sp, boundary = state["post"]
            emit_sums(sp)
            boundary()
            state["post"] = None

        def emit_sums(sp):
            sums, es, first, last = sp
            mm(sums[:, 0:512], onesb, es[:, 0:512], first, last)
            mm(sums[:, 512:1024], onesb, es[:, 512:1024], first, last)

        def flush():
            p = state["pending"]
            if p is None:
                return
            p0, p1, e, vt, first, last = p
            mm(p0, vt, e[:, 0:512], first, last)
            mm(p1, vt, e[:, 512:1024], first, last)
            state["pending"] = None

        def make_boundary(h, comp, sums, pp, recs, psbs):
            def boundary():
                lnt = pRec.tile([128, 1024], MDT, tag="rec", name=f"lnt{h}_{comp}")
                nc.scalar.activation(lnt, sums, func=LN)
                rec = pRec.tile([128, 1024], MDT, tag="rec", name=f"rec{h}_{comp}")
                nc.scalar.activation(rec, lnt, func=EXP, scale=-1.0)
                recs.append(rec)
                psb = pPsb.tile([128, 1024], MDT, tag="psb", name=f"psb{h}_{comp}")
                nc.vector.tensor_copy(psb, pp)
                psbs.append(psb)

            return boundary

        def emit_comp(h, comp, qs, qw, ks, kn, v2t, recs, psbs, filler=None):
            ks_t = kn if comp == 0 else ks
            qs_t = qs if comp == 0 else qw
            sums = ps_sums.tile([128, 1024], F32, tag="sums", name=f"sums{h}_{comp}")
            pp = ps_p.tile([128, 1024], F32, tag="p", name=f"pp{h}_{comp}")
            p0 = pp[:, 0:512]
            p1 = pp[:, 512:1024]
            e_prev = None
            sums_pend = None
            for tk in range(TC):
                st = ps_pair.tile([128, 1024], F32, tag="pair", name=f"st{h}_{comp}_{tk}")
                ksl = slice(tk * 128, (tk + 1) * 128)
                mm(st[:, 0:512], ks_t[:, ksl], qs_t[:, 0:512], True, True)
                mm(st[:, 512:1024], ks_t[:, ksl], qs_t[:, 512:1024], True, True)
                e = pE.tile([128, 1024], BF16, tag="e", name=f"e{h}_{comp}_{tk}")
                nc.scalar.activation(e, st, func=EXP)
                # bf16 pairwise e-sums on the DVE halve the ones-matmuls
                if tk % 2 == 1:
                    es = pEs.tile([128, 1024], BF16, tag="es", name=f"es{h}_{comp}_{tk}")
                    nc.vector.tensor_add(es, e_prev, e)
                if tk == 0 and filler is not None:
                    filler()
                if tk == 0 and state["post"] is not None:
                    flush_post()
                else:
                    flush()
                if tk % 2 == 1:
                    # lag-2: previous pair's sums matmuls go out now
                    if sums_pend is not None:
                        emit_sums(sums_pend)
                    sums_pend = (sums, es, tk == 1, tk == TC - 1)
                if comp == 0:
                    vt = v1[:, tk * 1024 + h * 128 : tk * 1024 + h * 128 + 128]
                else:
                    vt = v2t[:, tk * 128 : (tk + 1) * 128]
                state["pending"] = (p0, p1, e, vt, tk == 0, tk == TC - 1)
                e_prev = e
            state["post"] = (sums_pend, make_boundary(h, comp, sums, pp, recs, psbs))

        def emit_norm(h, recs, psbs):
            osl = slice(h * 1024, (h + 1) * 1024)
            t1 = pEs.tile([128, 1024], BF16, tag="es", name=f"t1_{h}")
            nc.vector.tensor_mul(t1, psbs[0], recs[0])
            nc.vector.tensor_mul(osb[:, osl], psbs[1], recs[1])
            nc.vector.tensor_add(osb[:, osl], osb[:, osl], t1)

        # prologue: head 0 projections
        prefetch_w(0)
        for kk in range(KC):
            tk_ = pB.tile([128, S], MDT, tag="bigB", name=f"xtk{kk}")
            dma_split(tk_, d_xtk[kk], n=2)
            xtk.append(tk_)
        prefetch_w(1)
        for kk in range(KC):
            tq = pA.tile([128, S], MDT, tag="bigA", name=f"xtq{kk}")
            dma_split(tq, d_xtq[kk], n=2)
            xtq.append(tq)
        kp = emit_k_chain(0)
        ks0, kn0 = finish_k(0, kp)
        qp = emit_q_chain(0)
        qs0, qw0 = finish_q(0, qp)
        cur = (qs0, qw0, ks0, kn0, build_v2(0))

        kp_box = {}
        for h in range(H):
            prefetch_w(h + 2)
            recs, psbs = [], []
            emit_comp(h, 0, *cur, recs, psbs)
            # splice the next head's K chain into the comp0->comp1 boundary
            # (PE filler while exp(c0,7) + ln/rec complete)
            filler = None
            if h + 1 < H:
                def filler(hh=h + 1):
                    kp_box["kp"] = emit_k_chain(hh)
            emit_comp(h, 1, *cur, recs, psbs, filler=filler)
            if h + 1 < H:
                ksn, knn = finish_k(h + 1, kp_box.pop("kp"))
                qp = emit_q_chain(h + 1)  # PE filler for the c1-iter7 flush
                flush_post()  # c1 final AV+sums + boundary (rec_i, P2 evac)
                qsn, qwn = finish_q(h + 1, qp)
                nxt = (qsn, qwn, ksn, knn, build_v2(h + 1))
            else:
                # last head: prefetch wo during the tail
                wo_t = []
                for hh in range(H):
                    tw = pA.tile([128, 1024], BF16, tag="bigA", name=f"wo{hh}")
                    nc.sync.dma_start(out=tw, in_=d_wo[hh])
                    wo_t.append(tw)
                flush_post()
                nxt = None
            emit_norm(h, recs, psbs)
            cur = nxt

        # ---- output projection (transposed: out[odim, tok]) ----
        for oc in range(KC):
            pair = ps_pair.tile([128, 1024], F32, tag="pair", name=f"ops{oc}")
            osl = slice(oc * 128, (oc + 1) * 128)
            for hh in range(H):
                lhsT = wo_t[hh][:, osl]
                hb = hh * 1024
                mm(pair[:, 0:512], lhsT, osb[:, hb : hb + 512], hh == 0, hh == H - 1)
                mm(
                    pair[:, 512:1024],
                    lhsT,
                    osb[:, hb + 512 : hb + 1024],
                    hh == 0,
                    hh == H - 1,
                )
            oev = pB.tile([128, 1024], F32, tag="bigB", name=f"oev{oc}")
            if oc % 2 == 0:
                nc.scalar.copy(oev, pair)
            else:
                nc.vector.tensor_copy(oev, pair)
            nc.sync.dma_start(out=d_out[oc][:, 0:512], in_=oev[:, 0:512])
            nc.sync.dma_start(out=d_out[oc][:, 512:1024], in_=oev[:, 512:1024])

    _split_waits(nc)
    return nc


_NC_CACHE = {}


def kernel(
    queries,
    keys,
    values,
    wq_r,
    wq_i,
    wk_r,
    wk_i,
    wv_r,
    wv_i,
    wo_r,
    wo_i,
    _trace=False,
):
    global LAST_EXEC_NS
    _install_axon_profile_shim()
    _install_tile_drain_patch()
    from concourse.bass_utils import run_bass_kernel_spmd

    scale = 1.0 / np.sqrt(DH)
    WQ = _head_tiles(_build_wqk(np.asarray(wq_r), np.asarray(wq_i), scale))
    WK = _head_tiles(_build_wqk(np.asarray(wk_r), np.asarray(wk_i), 1.0))
    import ml_dtypes
    WV = _kchunk_tiles(_build_wqk(np.asarray(wv_r), np.asarray(wv_i), 1.0)).astype(
        ml_dtypes.bfloat16
    )
    WO = _kchunk_tiles(_build_wo(np.asarray(wo_r), np.asarray(wo_i))).astype(
        ml_dtypes.bfloat16
    )
    CST = np.zeros((128, 320), np.float32)
    CST[:, 0:128] = 1.0
    CSTB = np.ones((128, 128), ml_dtypes.bfloat16)

    queries = np.asarray(queries)
    keys = np.asarray(keys)
    values = np.asarray(values)

    in_maps = []
    for b in range(NCORES):
        in_maps.append(
            {
                "xtq": _xt(queries[b]).reshape(KC, 128, S),
                "xtk": _xt(keys[b]).reshape(KC, 128, S),
                "xtv": _xt(values[b]).reshape(KC, 128, S).astype(ml_dtypes.bfloat16),
                "wq": WQ,
                "wk": WK,
                "wv": WV,
                "wo": WO,
                "cst": CST,
                "cstb": CSTB,
            }
        )

    if "nc" not in _NC_CACHE:
        _NC_CACHE["nc"] = _build_nc()
    nc = _NC_CACHE["nc"]

    res = run_bass_kernel_spmd(nc, in_maps, list(range(NCORES)), trace=_trace)
    LAST_EXEC_NS = res.exec_time_ns

    out = np.empty((B, S, D, 2), np.float32)
    for b in range(NCORES):
        # res: [oc, odim, tok] -> [tok, oc*128+odim] -> [S, D, 2]
        r = res.results[b]["out"].reshape(1024, S)
        out[b] = r.T.reshape(S, D, 2)
    return out


# revision 16
# speedup vs baseline: 1.2256x; 1.0075x over previous
"""Complex multi-head attention on 8 Trainium2 cores (Bass/Tile).

Sharding: pure data-parallel over batch (B=8 -> 1 batch per core),
weights replicated. No collectives.

v2 redesign (vs baseline at 465us):
  - Every LDWEIGHTS serves 2 matmuls (the two 512-col halves of each
    1024-wide rhs share the stationary operand) so the fp32 weight-load
    (224ns, no FWL for fp32) hides under 2x213ns of streaming.
  - Scores for both halves land in one 2-bank PSUM pair tile and are
    exponentiated with a single [128,1024] activation (amortizes the
    ~300-cycle ACT instruction overhead).
  - Softmax reciprocal moved off the DVE (was 32 x 3.9us of iterative
    divide) to the scalar engine as exp(-ln(sums)); Ln+Exp share one
    ACT table set.
  - P1/P2 evacuated unnormalized; normalization (3 DVE ops/head)
    happens off the critical path during the next head's projections.
  - Lag-1 software pipeline: st pair of iter i+1 is emitted between
    exp(i) and the sums/AV matmuls of iter i, so the PE never waits on
    the scalar engine; next head's K/Q projection chains are spliced
    into the head boundary to cover the last exp + rec latency.
  - O-projection computed transposed (out = [odim, tok]) with
    wo-stationary h-chains so its LDW is also paired; host undoes the
    transpose.
PSUM: st/proj/V/O pair pool [128,1024]x2 (4 banks) + sums pair
[128,1024]x1 (2) + P accumulators [128,512]x2 (2) = 8 banks exactly.
"""

import sys
import types
import numpy as np

B, S, D, H = 8, 1024, 512, 8
DH = D // H
KC = 8  # k-chunks of 128 over (c,d) = 1024
TC = 8  # token chunks of 128
NCORES = 8

LAST_EXEC_NS = None


# ---------------------------------------------------------------- shims
def _install_axon_profile_shim():
    if "antenv.axon_hooks" in sys.modules:
        return
    try:
        import antenv  # noqa: F401

        mod = types.ModuleType("antenv.axon_hooks")
        state = {"hook": None}
        mod.set_axon_ntff_profile_hook = lambda h: state.__setitem__("hook", h)
        mod.get_axon_ntff_profile_hook = lambda: state["hook"]
        sys.modules["antenv.axon_hooks"] = mod
        from trn_agent_boot.trn_boot import _ntff_profile_via_ctypes

        hook = _ntff_profile_via_ctypes("/opt/axon/libaxon_pjrt.so")
        if hook is not None:
            mod.set_axon_ntff_profile_hook(hook)
    except Exception:
        pass


def _install_tile_drain_patch():
    """This walrus build allows ONE sync wait per instruction; split the
    TileContext exit drain's waits across preceding sync NOPs."""
    import concourse.mybir as mybir
    import concourse.tile as tile
    from concourse.vector_clock import ScopedClock

    if getattr(tile.TileContext, "_drain_patched", False):
        return

    def _patched(self, tick_clock, wait_clock):
        probe = mybir.InstNoOp(name="I-drain-probe")
        probe.engine = mybir.EngineType.SP
        wait_clock.add_sem_waits(probe, ScopedClock({None: tick_clock.global_clock}))
        waits = list(probe.sync_info.on_wait or []) if probe.sync_info else []
        for w in waits:
            nop = self.nc.sync.nop()
            nop.ins.sync_info = mybir.SyncInfo(on_wait=[w], on_update=[])
        self.nc.sync.drain()
        self.nc.all_engine_barrier()
        assert self.sems is not None
        popped = self.nc._tile_sem_poison_stack.pop()
        assert popped is self._sem_poison
        self.nc.clear_and_free_semaphores(list(self.sems.allocated().values()))
        self.nc.all_engine_barrier()

    tile.TileContext._drain_and_barrier = _patched
    tile.TileContext._drain_patched = True


def _split_waits(nc, max_waits=1):
    """Hoist extra sync waits onto preceding same-engine NOPs (walrus here
    rejects >1 sync wait per instruction)."""
    import concourse.mybir as mybir

    def process(blk):
        lst = blk.instructions
        i = 0
        while i < len(lst):
            inst = lst[i]
            if hasattr(inst, "blocks"):
                for b in inst.blocks or []:
                    process(b)
            si = inst.sync_info
            if si is not None and si.on_wait and len(si.on_wait) > max_waits:
                waits = list(si.on_wait)
                keep, extra = waits[-max_waits:], waits[:-max_waits]
                inst.sync_info = mybir.SyncInfo(
                    on_wait=keep, on_update=list(si.on_update or [])
                )
                for j, w in enumerate(extra):
                    nop = mybir.InstNoOp(name=f"{inst.name}-ws{j}")
                    nop.engine = inst.engine
                    nop.sync_info = mybir.SyncInfo(on_wait=[w], on_update=[])
                    lst.insert(i, nop)
                    i += 1
            i += 1

    for f in nc.m.functions:
        for blk in f.blocks:
            process(blk)


# ------------------------------------------------------------ host prep
def _build_wqk(wr, wi, scale):
    """[1024 k=(c,d), 1024 m=(h, c', dh)] for Q/K projections."""
    W = np.empty((2 * D, 2 * D), np.float32)
    for h in range(H):
        o = slice(h * DH, (h + 1) * DH)
        c0 = h * 2 * DH
        W[0:D, c0 : c0 + DH] = wr[o].T * scale
        W[D:, c0 : c0 + DH] = -wi[o].T * scale
        W[0:D, c0 + DH : c0 + 2 * DH] = wi[o].T * scale
        W[D:, c0 + DH : c0 + 2 * DH] = wr[o].T * scale
    return W


def _head_tiles(W):
    """[1024,1024] -> [H, 128, 1024]: per-head column block, k-chunk cols."""
    out = np.empty((H, 128, 1024), np.float32)
    for h in range(H):
        blk = W[:, h * 128 : (h + 1) * 128]  # [1024, 128]
        for kk in range(KC):
            out[h, :, kk * 128 : (kk + 1) * 128] = blk[kk * 128 : (kk + 1) * 128]
    return out


def _kchunk_tiles(W):
    """[1024,1024] -> [KC, 128, 1024]: row chunks."""
    return np.ascontiguousarray(W.reshape(KC, 128, 1024))


def _build_wo(wo_r, wo_i):
    """rows (h, c', dh), cols (o, c) interleaved to match [S, D, 2]."""
    W = np.empty((2 * D, 2 * D), np.float32)
    for h in range(H):
        d = slice(h * DH, (h + 1) * DH)
        r0 = h * 2 * DH
        W[r0 : r0 + DH, 0::2] = wo_r[:, d].T
        W[r0 : r0 + DH, 1::2] = wo_i[:, d].T
        W[r0 + DH : r0 + 2 * DH, 0::2] = -wo_i[:, d].T
        W[r0 + DH : r0 + 2 * DH, 1::2] = wo_r[:, d].T
    return W


def _xt(x):  # [S, D, 2] -> [2D, S] feature-major
    out = np.empty((2 * D, S), np.float32)
    out[0:D] = x[:, :, 0].T
    out[D:] = x[:, :, 1].T
    return out


# ------------------------------------------------------------ bass build
def _build_nc():
    import concourse.bass as bass
    import concourse.mybir as mybir
    import concourse.tile as tile
    from contextlib import ExitStack

    MDT = mybir.dt.float32r
    F32 = mybir.dt.float32
    BF16 = mybir.dt.bfloat16
    EXP = mybir.ActivationFunctionType.Exp
    LN = mybir.ActivationFunctionType.Ln

    nc = bass.Bass()
    d_xtq = nc.dram_tensor("xtq", [KC, 128, S], MDT, kind="ExternalInput")
    d_xtk = nc.dram_tensor("xtk", [KC, 128, S], MDT, kind="ExternalInput")
    d_xtv = nc.dram_tensor("xtv", [KC, 128, S], BF16, kind="ExternalInput")
    d_wq = nc.dram_tensor("wq", [H, 128, 1024], MDT, kind="ExternalInput")
    d_wk = nc.dram_tensor("wk", [H, 128, 1024], MDT, kind="ExternalInput")
    d_wv = nc.dram_tensor("wv", [KC, 128, 1024], BF16, kind="ExternalInput")
    d_wo = nc.dram_tensor("wo", [H, 128, 1024], BF16, kind="ExternalInput")
    d_cst = nc.dram_tensor("cst", [128, 320], MDT, kind="ExternalInput")
    d_cstb = nc.dram_tensor("cstb", [128, 128], BF16, kind="ExternalInput")
    # transposed output: [oc, odim, tok]
    d_out = nc.dram_tensor("out", [KC, 128, S], BF16, kind="ExternalOutput")

    with tile.TileContext(nc) as tc, ExitStack() as ctx:
        ctx.enter_context(
            nc.allow_low_precision(reason="float32r tiles are bit-identical fp32")
        )
        pA = ctx.enter_context(tc.tile_pool(name="bigA", bufs=8))  # xtv->xtq->wo
        pB = ctx.enter_context(tc.tile_pool(name="bigB", bufs=8))  # wv->xtk->oev
        pV1 = ctx.enter_context(tc.tile_pool(name="v1", bufs=1))
        pOsb = ctx.enter_context(tc.tile_pool(name="osb", bufs=1))
        pQs = ctx.enter_context(tc.tile_pool(name="qs", bufs=2))
        pKs = ctx.enter_context(tc.tile_pool(name="ks", bufs=2))
        pQw = ctx.enter_context(tc.tile_pool(name="qw", bufs=1))
        pKn = ctx.enter_context(tc.tile_pool(name="kn", bufs=1))
        pV2 = ctx.enter_context(tc.tile_pool(name="v2", bufs=1))
        pE = ctx.enter_context(tc.tile_pool(name="e", bufs=6))
        pEs = ctx.enter_context(tc.tile_pool(name="es", bufs=6))
        pPsb = ctx.enter_context(tc.tile_pool(name="psb", bufs=3))
        pRec = ctx.enter_context(tc.tile_pool(name="rec", bufs=4))
        pWqk = ctx.enter_context(tc.tile_pool(name="wqk", bufs=4))
        pC = ctx.enter_context(tc.tile_pool(name="const", bufs=1))

        ps_pair = ctx.enter_context(tc.tile_pool(name="ps_pair", bufs=2, space="PSUM"))
        ps_sums = ctx.enter_context(tc.tile_pool(name="ps_sums", bufs=1, space="PSUM"))
        ps_p = ctx.enter_context(tc.tile_pool(name="ps_p", bufs=1, space="PSUM"))

        cst = pC.tile([128, 320], MDT, tag="cst")
        nc.sync.dma_start(out=cst, in_=d_cst[:, :])
        onesb = pC.tile([128, 128], BF16, tag="cstb")
        nc.sync.dma_start(out=onesb, in_=d_cstb[:, :])

        def mm(out, lhsT, rhs, start, stop):
            nc.tensor.matmul(out, lhsT=lhsT, rhs=rhs, start=start, stop=stop)

        # ---- phase V: V projection (all heads) ----
        def dma_split(out, in_, n=4):
            w = out.shape[-1] // n
            for i in range(n):
                nc.sync.dma_start(
                    out=out[:, i * w : (i + 1) * w], in_=in_[:, i * w : (i + 1) * w]
                )

        xtv, wv = [], []
        for kk in range(KC):
            tv = pA.tile([128, S], BF16, tag="bigA", name=f"xtv{kk}")
            nc.sync.dma_start(out=tv, in_=d_xtv[kk])
            xtv.append(tv)
            tw = pB.tile([128, 1024], BF16, tag="bigB", name=f"wv{kk}")
            nc.sync.dma_start(out=tw, in_=d_wv[kk])
            wv.append(tw)

        v1 = pV1.tile([128, TC * 1024], BF16, tag="v1")  # [p, (t_, h, dh2)]
        for t_ in range(TC):
            pair = ps_pair.tile([128, 1024], F32, tag="pair", name=f"vps{t_}")
            tsl = slice(t_ * 128, (t_ + 1) * 128)
            for kk in range(KC):
                lhsT = xtv[kk][:, tsl]
                mm(pair[:, 0:512], lhsT, wv[kk][:, 0:512], kk == 0, kk == KC - 1)
                mm(pair[:, 512:1024], lhsT, wv[kk][:, 512:1024], kk == 0, kk == KC - 1)
            nc.vector.tensor_copy(v1[:, t_ * 1024 : (t_ + 1) * 1024], pair)

        # ---- load XT_q / XT_k (reuse pA / pB slots) ----
        xtq, xtk = [], []

        osb = pOsb.tile([128, H * 1024], BF16, tag="osb")  # [p, (h, tok)]

        # ---------------- per-head helpers ----------------
        wq_t, wk_t = {}, {}

        def prefetch_w(h):
            if h >= H or h in wq_t:
                return
            tk2 = pWqk.tile([128, 1024], MDT, tag="wqk", name=f"wk{h}")
            dma_split(tk2, d_wk[h], n=2)
            wk_t[h] = tk2
            tq = pWqk.tile([128, 1024], MDT, tag="wqk", name=f"wq{h}")
            dma_split(tq, d_wq[h], n=2)
            wq_t[h] = tq

        def proj_chain(w_tile, xt_tiles, name):
            pair = ps_pair.tile([128, 1024], F32, tag="pair", name=name)
            for kk in range(KC):
                lhsT = w_tile[:, kk * 128 : (kk + 1) * 128]
                mm(pair[:, 0:512], lhsT, xt_tiles[kk][:, 0:512], kk == 0, kk == KC - 1)
                mm(
                    pair[:, 512:1024],
                    lhsT,
                    xt_tiles[kk][:, 512:1024],
                    kk == 0,
                    kk == KC - 1,
                )
            return pair

        def emit_k_chain(h):
            return proj_chain(wk_t.pop(h), xtk, f"kproj{h}")

        def finish_k(h, pair):
            ks = pKs.tile([128, S], MDT, tag="ks", name=f"kstack{h}")
            nc.vector.tensor_copy(ks, pair)
            kn = pKn.tile([128, S], MDT, tag="kn", name=f"kneg{h}")
            nc.vector.tensor_copy(kn[0:64, :], ks[0:64, :])
            nc.vector.tensor_scalar_mul(kn[64:128, :], ks[64:128, :], -1.0)
            return ks, kn

        def emit_q_chain(h):
            return proj_chain(wq_t.pop(h), xtq, f"qproj{h}")

        def finish_q(h, pair):
            qs = pQs.tile([128, S], MDT, tag="qs", name=f"qstack{h}")
            nc.vector.tensor_copy(qs[:, 0:512], pair[:, 0:512])
            nc.vector.tensor_copy(qs[:, 512:1024], pair[:, 512:1024])
            qw = pQw.tile([128, S], MDT, tag="qw", name=f"qswap{h}")
            nc.sync.dma_start(out=qw[0:64, :], in_=qs[64:128, :])
            nc.sync.dma_start(out=qw[64:128, :], in_=qs[0:64, :])
            return qs, qw

        def build_v2(h):
            v2t = pV2.tile([128, 1024], BF16, tag="v2", name=f"v2h{h}")
            v1v = v1.rearrange("p (t h d) -> p t h d", t=TC, h=H, d=128)
            v2v = v2t.rearrange("p (t d) -> p t d", t=TC, d=128)
            nc.vector.tensor_scalar_mul(
                v2v[:, :, 0:64], v1v[:, :, h, 64:128], -1.0
            )
            nc.vector.tensor_copy(v2v[:, :, 64:128], v1v[:, :, h, 0:64])
            return v2t

        # ---------------- the pipelined head loop ----------------
        # pending = (sums, p0, p1, e, vt, first, last, boundary_cb)
        state = {"pending": None, "post": None}

        def flush_post():
            flush()
            sp, boundary = state["post"]
            emit_sums(sp)
            boundary()
            state["post"] = None

        def emit_sums(sp):
            sums, es, first, last = sp
            mm(sums[:, 0:512], onesb, es[:, 0:512], first, last)
            mm(sums[:, 512:1024], onesb, es[:, 512:1024], first, last)

        def flush():
            p = state["pending"]
            if p is None:
                return
            p0, p1, e, vt, first, last = p
            mm(p0, vt, e[:, 0:512], first, last)
            mm(p1, vt, e[:, 512:1024], first, last)
            state["pending"] = None

        def make_boundary(h, comp, sums, pp, recs, psbs):
            def boundary():
                lnt = pRec.tile([128, 1024], MDT, tag="rec", name=f"lnt{h}_{comp}")
                nc.scalar.activation(lnt, sums, func=LN)
                rec = pRec.tile([128, 1024], MDT, tag="rec", name=f"rec{h}_{comp}")
                nc.scalar.activation(rec, lnt, func=EXP, scale=-1.0)
                recs.append(rec)
                psb = pPsb.tile([128, 1024], MDT, tag="psb", name=f"psb{h}_{comp}")
                nc.vector.tensor_copy(psb, pp)
                psbs.append(psb)

            return boundary

        def emit_comp(h, comp, qs, qw, ks, kn, v2t, recs, psbs, filler=None):
            ks_t = kn if comp == 0 else ks
            qs_t = qs if comp == 0 else qw
            sums = ps_sums.tile([128, 1024], F32, tag="sums", name=f"sums{h}_{comp}")
            pp = ps_p.tile([128, 1024], F32, tag="p", name=f"pp{h}_{comp}")
            p0 = pp[:, 0:512]
            p1 = pp[:, 512:1024]
            e_prev = None
            sums_pend = None
            for tk in range(TC):
                st = ps_pair.tile([128, 1024], F32, tag="pair", name=f"st{h}_{comp}_{tk}")
                ksl = slice(tk * 128, (tk + 1) * 128)
                mm(st[:, 0:512], ks_t[:, ksl], qs_t[:, 0:512], True, True)
                mm(st[:, 512:1024], ks_t[:, ksl], qs_t[:, 512:1024], True, True)
                e = pE.tile([128, 1024], BF16, tag="e", name=f"e{h}_{comp}_{tk}")
                nc.scalar.activation(e, st, func=EXP)
                # bf16 pairwise e-sums on the DVE halve the ones-matmuls
                if tk % 2 == 1:
                    es = pEs.tile([128, 1024], BF16, tag="es", name=f"es{h}_{comp}_{tk}")
                    nc.vector.tensor_add(es, e_prev, e)
                if tk == 0 and filler is not None:
                    filler()
                if tk == 0 and state["post"] is not None:
                    flush_post()
                else:
                    flush()
                if tk % 2 == 1:
                    # lag-2: previous pair's sums matmuls go out now
                    if sums_pend is not None:
                        emit_sums(sums_pend)
                    sums_pend = (sums, es, tk == 1, tk == TC - 1)
                if comp == 0:
                    vt = v1[:, tk * 1024 + h * 128 : tk * 1024 + h * 128 + 128]
                else:
                    vt = v2t[:, tk * 128 : (tk + 1) * 128]
                state["pending"] = (p0, p1, e, vt, tk == 0, tk == TC - 1)
                e_prev = e
            state["post"] = (sums_pend, make_boundary(h, comp, sums, pp, recs, psbs))

        def emit_norm(h, recs, psbs):
            osl = slice(h * 1024, (h + 1) * 1024)
            t1 = pEs.tile([128, 1024], BF16, tag="es", name=f"t1_{h}")
            nc.vector.tensor_mul(t1, psbs[0], recs[0])
            nc.vector.tensor_mul(osb[:, osl], psbs[1], recs[1])
            nc.vector.tensor_add(osb[:, osl], osb[:, osl], t1)

        # prologue: head 0 projections
        prefetch_w(0)
        for kk in range(KC):
            tk_ = pB.tile([128, S], MDT, tag="bigB", name=f"xtk{kk}")
            dma_split(tk_, d_xtk[kk], n=2)
            xtk.append(tk_)
        prefetch_w(1)
        for kk in range(KC):
            tq = pA.tile([128, S], MDT, tag="bigA", name=f"xtq{kk}")
            dma_split(tq, d_xtq[kk], n=2)
            xtq.append(tq)
        kp = emit_k_chain(0)
        ks0, kn0 = finish_k(0, kp)
        qp = emit_q_chain(0)
        qs0, qw0 = finish_q(0, qp)
        cur = (qs0, qw0, ks0, kn0, build_v2(0))

        kp_box = {}
        for h in range(H):
            prefetch_w(h + 2)
            recs, psbs = [], []
            emit_comp(h, 0, *cur, recs, psbs)
            # splice the next head's K chain into the comp0->comp1 boundary
            # (PE filler while exp(c0,7) + ln/rec complete)
            filler = None
            if h + 1 < H:
                def filler(hh=h + 1):
                    kp_box["kp"] = emit_k_chain(hh)
            emit_comp(h, 1, *cur, recs, psbs, filler=filler)
            if h + 1 < H:
                ksn, knn = finish_k(h + 1, kp_box.pop("kp"))
                qp = emit_q_chain(h + 1)  # PE filler for the c1-iter7 flush
                flush_post()  # c1 final AV+sums + boundary (rec_i, P2 evac)
                qsn, qwn = finish_q(h + 1, qp)
                nxt = (qsn, qwn, ksn, knn, build_v2(h + 1))
            else:
                # last head: prefetch wo during the tail
                wo_t = []
                for hh in range(H):
                    tw = pA.tile([128, 1024], BF16, tag="bigA", name=f"wo{hh}")
                    nc.sync.dma_start(out=tw, in_=d_wo[hh])
                    wo_t.append(tw)
                flush_post()
                nxt = None
            emit_norm(h, recs, psbs)
            cur = nxt

        # ---- output projection (transposed: out[odim, tok]) ----
        for oc in range(KC):
            pair = ps_pair.tile([128, 1024], F32, tag="pair", name=f"ops{oc}")
            osl = slice(oc * 128, (oc + 1) * 128)
            for hh in range(H):
                lhsT = wo_t[hh][:, osl]
                hb = hh * 1024
                mm(pair[:, 0:512], lhsT, osb[:, hb : hb + 512], hh == 0, hh == H - 1)
                mm(
                    pair[:, 512:1024],
                    lhsT,
                    osb[:, hb + 512 : hb + 1024],
                    hh == 0,
                    hh == H - 1,
                )
            oev = pB.tile([128, 1024], BF16, tag="bigB", name=f"oev{oc}")
            if oc % 2 == 0:
                nc.scalar.copy(oev, pair)
            else:
                nc.vector.tensor_copy(oev, pair)
            nc.sync.dma_start(out=d_out[oc][:, 0:512], in_=oev[:, 0:512])
            nc.sync.dma_start(out=d_out[oc][:, 512:1024], in_=oev[:, 512:1024])

    _split_waits(nc)
    return nc


_NC_CACHE = {}


def kernel(
    queries,
    keys,
    values,
    wq_r,
    wq_i,
    wk_r,
    wk_i,
    wv_r,
    wv_i,
    wo_r,
    wo_i,
    _trace=False,
):
    global LAST_EXEC_NS
    _install_axon_profile_shim()
    _install_tile_drain_patch()
    from concourse.bass_utils import run_bass_kernel_spmd

    scale = 1.0 / np.sqrt(DH)
    WQ = _head_tiles(_build_wqk(np.asarray(wq_r), np.asarray(wq_i), scale))
    WK = _head_tiles(_build_wqk(np.asarray(wk_r), np.asarray(wk_i), 1.0))
    import ml_dtypes
    WV = _kchunk_tiles(_build_wqk(np.asarray(wv_r), np.asarray(wv_i), 1.0)).astype(
        ml_dtypes.bfloat16
    )
    WO = _kchunk_tiles(_build_wo(np.asarray(wo_r), np.asarray(wo_i))).astype(
        ml_dtypes.bfloat16
    )
    CST = np.zeros((128, 320), np.float32)
    CST[:, 0:128] = 1.0
    CSTB = np.ones((128, 128), ml_dtypes.bfloat16)

    queries = np.asarray(queries)
    keys = np.asarray(keys)
    values = np.asarray(values)

    in_maps = []
    for b in range(NCORES):
        in_maps.append(
            {
                "xtq": _xt(queries[b]).reshape(KC, 128, S),
                "xtk": _xt(keys[b]).reshape(KC, 128, S),
                "xtv": _xt(values[b]).reshape(KC, 128, S).astype(ml_dtypes.bfloat16),
                "wq": WQ,
                "wk": WK,
                "wv": WV,
                "wo": WO,
                "cst": CST,
                "cstb": CSTB,
            }
        )

    if "nc" not in _NC_CACHE:
        _NC_CACHE["nc"] = _build_nc()
    nc = _NC_CACHE["nc"]

    res = run_bass_kernel_spmd(nc, in_maps, list(range(NCORES)), trace=_trace)
    LAST_EXEC_NS = res.exec_time_ns

    out = np.empty((B, S, D, 2), np.float32)
    for b in range(NCORES):
        # res: [oc, odim, tok] -> [tok, oc*128+odim] -> [S, D, 2]
        r = np.asarray(res.results[b]["out"], np.float32).reshape(1024, S)
        out[b] = r.T.reshape(S, D, 2)
    return out


# revision 17
# speedup vs baseline: 1.2363x; 1.0087x over previous
"""Complex multi-head attention on 8 Trainium2 cores (Bass/Tile).

Sharding: pure data-parallel over batch (B=8 -> 1 batch per core),
weights replicated. No collectives.

v2 redesign (vs baseline at 465us):
  - Every LDWEIGHTS serves 2 matmuls (the two 512-col halves of each
    1024-wide rhs share the stationary operand) so the fp32 weight-load
    (224ns, no FWL for fp32) hides under 2x213ns of streaming.
  - Scores for both halves land in one 2-bank PSUM pair tile and are
    exponentiated with a single [128,1024] activation (amortizes the
    ~300-cycle ACT instruction overhead).
  - Softmax reciprocal moved off the DVE (was 32 x 3.9us of iterative
    divide) to the scalar engine as exp(-ln(sums)); Ln+Exp share one
    ACT table set.
  - P1/P2 evacuated unnormalized; normalization (3 DVE ops/head)
    happens off the critical path during the next head's projections.
  - Lag-1 software pipeline: st pair of iter i+1 is emitted between
    exp(i) and the sums/AV matmuls of iter i, so the PE never waits on
    the scalar engine; next head's K/Q projection chains are spliced
    into the head boundary to cover the last exp + rec latency.
  - O-projection computed transposed (out = [odim, tok]) with
    wo-stationary h-chains so its LDW is also paired; host undoes the
    transpose.
PSUM: st/proj/V/O pair pool [128,1024]x2 (4 banks) + sums pair
[128,1024]x1 (2) + P accumulators [128,512]x2 (2) = 8 banks exactly.
"""

import sys
import types
import numpy as np

B, S, D, H = 8, 1024, 512, 8
DH = D // H
KC = 8  # k-chunks of 128 over (c,d) = 1024
TC = 8  # token chunks of 128
NCORES = 8

LAST_EXEC_NS = None


# ---------------------------------------------------------------- shims
def _install_axon_profile_shim():
    if "antenv.axon_hooks" in sys.modules:
        return
    try:
        import antenv  # noqa: F401

        mod = types.ModuleType("antenv.axon_hooks")
        state = {"hook": None}
        mod.set_axon_ntff_profile_hook = lambda h: state.__setitem__("hook", h)
        mod.get_axon_ntff_profile_hook = lambda: state["hook"]
        sys.modules["antenv.axon_hooks"] = mod
        from trn_agent_boot.trn_boot import _ntff_profile_via_ctypes

        hook = _ntff_profile_via_ctypes("/opt/axon/libaxon_pjrt.so")
        if hook is not None:
            mod.set_axon_ntff_profile_hook(hook)
    except Exception:
        pass


def _install_tile_drain_patch():
    """This walrus build allows ONE sync wait per instruction; split the
    TileContext exit drain's waits across preceding sync NOPs."""
    import concourse.mybir as mybir
    import concourse.tile as tile
    from concourse.vector_clock import ScopedClock

    if getattr(tile.TileContext, "_drain_patched", False):
        return

    def _patched(self, tick_clock, wait_clock):
        probe = mybir.InstNoOp(name="I-drain-probe")
        probe.engine = mybir.EngineType.SP
        wait_clock.add_sem_waits(probe, ScopedClock({None: tick_clock.global_clock}))
        waits = list(probe.sync_info.on_wait or []) if probe.sync_info else []
        for w in waits:
            nop = self.nc.sync.nop()
            nop.ins.sync_info = mybir.SyncInfo(on_wait=[w], on_update=[])
        self.nc.sync.drain()
        self.nc.all_engine_barrier()
        assert self.sems is not None
        popped = self.nc._tile_sem_poison_stack.pop()
        assert popped is self._sem_poison
        self.nc.clear_and_free_semaphores(list(self.sems.allocated().values()))
        self.nc.all_engine_barrier()

    tile.TileContext._drain_and_barrier = _patched
    tile.TileContext._drain_patched = True


def _split_waits(nc, max_waits=1):
    """Hoist extra sync waits onto preceding same-engine NOPs (walrus here
    rejects >1 sync wait per instruction)."""
    import concourse.mybir as mybir

    def process(blk):
        lst = blk.instructions
        i = 0
        while i < len(lst):
            inst = lst[i]
            if hasattr(inst, "blocks"):
                for b in inst.blocks or []:
                    process(b)
            si = inst.sync_info
            if si is not None and si.on_wait and len(si.on_wait) > max_waits:
                waits = list(si.on_wait)
                keep, extra = waits[-max_waits:], waits[:-max_waits]
                inst.sync_info = mybir.SyncInfo(
                    on_wait=keep, on_update=list(si.on_update or [])
                )
                for j, w in enumerate(extra):
                    nop = mybir.InstNoOp(name=f"{inst.name}-ws{j}")
                    nop.engine = inst.engine
                    nop.sync_info = mybir.SyncInfo(on_wait=[w], on_update=[])
                    lst.insert(i, nop)
                    i += 1
            i += 1

    for f in nc.m.functions:
        for blk in f.blocks:
            process(blk)


# ------------------------------------------------------------ host prep
def _build_wqk(wr, wi, scale):
    """[1024 k=(c,d), 1024 m=(h, c', dh)] for Q/K projections."""
    W = np.empty((2 * D, 2 * D), np.float32)
    for h in range(H):
        o = slice(h * DH, (h + 1) * DH)
        c0 = h * 2 * DH
        W[0:D, c0 : c0 + DH] = wr[o].T * scale
        W[D:, c0 : c0 + DH] = -wi[o].T * scale
        W[0:D, c0 + DH : c0 + 2 * DH] = wi[o].T * scale
        W[D:, c0 + DH : c0 + 2 * DH] = wr[o].T * scale
    return W


def _head_tiles(W):
    """[1024,1024] -> [H, 128, 1024]: per-head column block, k-chunk cols."""
    out = np.empty((H, 128, 1024), np.float32)
    for h in range(H):
        blk = W[:, h * 128 : (h + 1) * 128]  # [1024, 128]
        for kk in range(KC):
            out[h, :, kk * 128 : (kk + 1) * 128] = blk[kk * 128 : (kk + 1) * 128]
    return out


def _kchunk_tiles(W):
    """[1024,1024] -> [KC, 128, 1024]: row chunks."""
    return np.ascontiguousarray(W.reshape(KC, 128, 1024))


def _build_wo(wo_r, wo_i):
    """rows (h, c', dh), cols (o, c) interleaved to match [S, D, 2]."""
    W = np.empty((2 * D, 2 * D), np.float32)
    for h in range(H):
        d = slice(h * DH, (h + 1) * DH)
        r0 = h * 2 * DH
        W[r0 : r0 + DH, 0::2] = wo_r[:, d].T
        W[r0 : r0 + DH, 1::2] = wo_i[:, d].T
        W[r0 + DH : r0 + 2 * DH, 0::2] = -wo_i[:, d].T
        W[r0 + DH : r0 + 2 * DH, 1::2] = wo_r[:, d].T
    return W


def _xt(x):  # [S, D, 2] -> [2D, S] feature-major
    out = np.empty((2 * D, S), np.float32)
    out[0:D] = x[:, :, 0].T
    out[D:] = x[:, :, 1].T
    return out


# ------------------------------------------------------------ bass build
def _build_nc():
    import concourse.bass as bass
    import concourse.mybir as mybir
    import concourse.tile as tile
    from contextlib import ExitStack

    MDT = mybir.dt.float32r
    F32 = mybir.dt.float32
    BF16 = mybir.dt.bfloat16
    EXP = mybir.ActivationFunctionType.Exp
    LN = mybir.ActivationFunctionType.Ln

    nc = bass.Bass()
    d_xtq = nc.dram_tensor("xtq", [KC, 128, S], MDT, kind="ExternalInput")
    d_xtk = nc.dram_tensor("xtk", [KC, 128, S], MDT, kind="ExternalInput")
    d_xtv = nc.dram_tensor("xtv", [KC, 128, S], BF16, kind="ExternalInput")
    d_wq = nc.dram_tensor("wq", [H, 128, 1024], MDT, kind="ExternalInput")
    d_wk = nc.dram_tensor("wk", [H, 128, 1024], MDT, kind="ExternalInput")
    d_wv = nc.dram_tensor("wv", [KC, 128, 1024], BF16, kind="ExternalInput")
    d_wo = nc.dram_tensor("wo", [H, 128, 1024], BF16, kind="ExternalInput")
    d_cst = nc.dram_tensor("cst", [128, 320], MDT, kind="ExternalInput")
    d_cstb = nc.dram_tensor("cstb", [128, 128], BF16, kind="ExternalInput")
    # transposed output: [oc, odim, tok]
    d_out = nc.dram_tensor("out", [KC, 128, S], BF16, kind="ExternalOutput")

    with tile.TileContext(nc) as tc, ExitStack() as ctx:
        ctx.enter_context(
            nc.allow_low_precision(reason="float32r tiles are bit-identical fp32")
        )
        pA = ctx.enter_context(tc.tile_pool(name="bigA", bufs=8))  # xtv->xtq->wo
        pB = ctx.enter_context(tc.tile_pool(name="bigB", bufs=8))  # wv->xtk->oev
        pV1 = ctx.enter_context(tc.tile_pool(name="v1", bufs=1))
        pOsb = ctx.enter_context(tc.tile_pool(name="osb", bufs=1))
        pQs = ctx.enter_context(tc.tile_pool(name="qs", bufs=2))
        pKs = ctx.enter_context(tc.tile_pool(name="ks", bufs=2))
        pQw = ctx.enter_context(tc.tile_pool(name="qw", bufs=1))
        pKn = ctx.enter_context(tc.tile_pool(name="kn", bufs=1))
        pV2 = ctx.enter_context(tc.tile_pool(name="v2", bufs=1))
        pE = ctx.enter_context(tc.tile_pool(name="e", bufs=6))
        pEs = ctx.enter_context(tc.tile_pool(name="es", bufs=6))
        pPsb = ctx.enter_context(tc.tile_pool(name="psb", bufs=3))
        pRec = ctx.enter_context(tc.tile_pool(name="rec", bufs=4))
        pWqk = ctx.enter_context(tc.tile_pool(name="wqk", bufs=4))
        pC = ctx.enter_context(tc.tile_pool(name="const", bufs=1))

        ps_pair = ctx.enter_context(tc.tile_pool(name="ps_pair", bufs=2, space="PSUM"))
        ps_sums = ctx.enter_context(tc.tile_pool(name="ps_sums", bufs=1, space="PSUM"))
        ps_p = ctx.enter_context(tc.tile_pool(name="ps_p", bufs=1, space="PSUM"))

        cst = pC.tile([128, 320], MDT, tag="cst")
        nc.sync.dma_start(out=cst, in_=d_cst[:, :])
        onesb = pC.tile([128, 128], BF16, tag="cstb")
        nc.sync.dma_start(out=onesb, in_=d_cstb[:, :])

        def mm(out, lhsT, rhs, start, stop):
            nc.tensor.matmul(out, lhsT=lhsT, rhs=rhs, start=start, stop=stop)

        # ---- phase V: V projection (all heads) ----
        def dma_split(out, in_, n=4):
            w = out.shape[-1] // n
            for i in range(n):
                nc.sync.dma_start(
                    out=out[:, i * w : (i + 1) * w], in_=in_[:, i * w : (i + 1) * w]
                )

        xtv, wv = [], []
        for kk in range(KC):
            tv = pA.tile([128, S], BF16, tag="bigA", name=f"xtv{kk}")
            nc.sync.dma_start(out=tv, in_=d_xtv[kk])
            xtv.append(tv)
            tw = pB.tile([128, 1024], BF16, tag="bigB", name=f"wv{kk}")
            nc.sync.dma_start(out=tw, in_=d_wv[kk])
            wv.append(tw)

        v1 = pV1.tile([128, TC * 1024], BF16, tag="v1")  # [p, (t_, h, dh2)]
        for t_ in range(TC):
            pair = ps_pair.tile([128, 1024], F32, tag="pair", name=f"vps{t_}")
            tsl = slice(t_ * 128, (t_ + 1) * 128)
            for kk in range(KC):
                lhsT = xtv[kk][:, tsl]
                mm(pair[:, 0:512], lhsT, wv[kk][:, 0:512], kk == 0, kk == KC - 1)
                mm(pair[:, 512:1024], lhsT, wv[kk][:, 512:1024], kk == 0, kk == KC - 1)
            nc.vector.tensor_copy(v1[:, t_ * 1024 : (t_ + 1) * 1024], pair)

        # ---- load XT_q / XT_k (reuse pA / pB slots) ----
        xtq, xtk = [], []

        osb = pOsb.tile([128, H * 1024], BF16, tag="osb")  # [p, (h, tok)]

        # ---------------- per-head helpers ----------------
        wq_t, wk_t = {}, {}

        def prefetch_w(h):
            if h >= H or h in wq_t:
                return
            tk2 = pWqk.tile([128, 1024], MDT, tag="wqk", name=f"wk{h}")
            dma_split(tk2, d_wk[h], n=2)
            wk_t[h] = tk2
            tq = pWqk.tile([128, 1024], MDT, tag="wqk", name=f"wq{h}")
            dma_split(tq, d_wq[h], n=2)
            wq_t[h] = tq

        def proj_chain(w_tile, xt_tiles, name):
            pair = ps_pair.tile([128, 1024], F32, tag="pair", name=name)
            for kk in range(KC):
                lhsT = w_tile[:, kk * 128 : (kk + 1) * 128]
                mm(pair[:, 0:512], lhsT, xt_tiles[kk][:, 0:512], kk == 0, kk == KC - 1)
                mm(
                    pair[:, 512:1024],
                    lhsT,
                    xt_tiles[kk][:, 512:1024],
                    kk == 0,
                    kk == KC - 1,
                )
            return pair

        def emit_k_chain(h):
            return proj_chain(wk_t.pop(h), xtk, f"kproj{h}")

        def finish_k(h, pair):
            ks = pKs.tile([128, S], MDT, tag="ks", name=f"kstack{h}")
            nc.vector.tensor_copy(ks, pair)
            kn = pKn.tile([128, S], MDT, tag="kn", name=f"kneg{h}")
            nc.vector.tensor_copy(kn[0:64, :], ks[0:64, :])
            nc.vector.tensor_scalar_mul(kn[64:128, :], ks[64:128, :], -1.0)
            return ks, kn

        def emit_q_chain(h):
            return proj_chain(wq_t.pop(h), xtq, f"qproj{h}")

        def finish_q(h, pair):
            qs = pQs.tile([128, S], MDT, tag="qs", name=f"qstack{h}")
            nc.vector.tensor_copy(qs[:, 0:512], pair[:, 0:512])
            nc.vector.tensor_copy(qs[:, 512:1024], pair[:, 512:1024])
            qw = pQw.tile([128, S], MDT, tag="qw", name=f"qswap{h}")
            nc.sync.dma_start(out=qw[0:64, :], in_=qs[64:128, :])
            nc.sync.dma_start(out=qw[64:128, :], in_=qs[0:64, :])
            return qs, qw

        def build_v2(h):
            v2t = pV2.tile([128, 1024], BF16, tag="v2", name=f"v2h{h}")
            v1v = v1.rearrange("p (t h d) -> p t h d", t=TC, h=H, d=128)
            v2v = v2t.rearrange("p (t d) -> p t d", t=TC, d=128)
            nc.vector.tensor_scalar_mul(
                v2v[:, :, 0:64], v1v[:, :, h, 64:128], -1.0
            )
            nc.vector.tensor_copy(v2v[:, :, 64:128], v1v[:, :, h, 0:64])
            return v2t

        # ---------------- the pipelined head loop ----------------
        # pending = (sums, p0, p1, e, vt, first, last, boundary_cb)
        state = {"pending": None, "post": None}

        def flush_post():
            flush()
            sp, boundary = state["post"]
            emit_sums(sp)
            boundary()
            state["post"] = None

        def emit_sums(sp):
            sums, es, first, last = sp
            mm(sums[:, 0:512], onesb, es[:, 0:512], first, last)
            mm(sums[:, 512:1024], onesb, es[:, 512:1024], first, last)

        def flush():
            p = state["pending"]
            if p is None:
                return
            p0, p1, e, vt, first, last = p
            mm(p0, vt, e[:, 0:512], first, last)
            mm(p1, vt, e[:, 512:1024], first, last)
            state["pending"] = None

        def make_boundary(h, comp, sums, pp, recs, psbs):
            def boundary():
                lnt = pRec.tile([128, 1024], MDT, tag="rec", name=f"lnt{h}_{comp}")
                nc.scalar.activation(lnt, sums, func=LN)
                rec = pRec.tile([128, 1024], MDT, tag="rec", name=f"rec{h}_{comp}")
                nc.scalar.activation(rec, lnt, func=EXP, scale=-1.0)
                recs.append(rec)
                psb = pPsb.tile([128, 1024], MDT, tag="psb", name=f"psb{h}_{comp}")
                nc.vector.tensor_copy(psb, pp)
                psbs.append(psb)

            return boundary

        def emit_comp(h, comp, qs, qw, ks, kn, v2t, recs, psbs, filler=None):
            ks_t = kn if comp == 0 else ks
            qs_t = qs if comp == 0 else qw
            sums = ps_sums.tile([128, 1024], F32, tag="sums", name=f"sums{h}_{comp}")
            pp = ps_p.tile([128, 1024], F32, tag="p", name=f"pp{h}_{comp}")
            p0 = pp[:, 0:512]
            p1 = pp[:, 512:1024]
            e_prev = None
            sums_pend = None
            for tk in range(TC):
                st = ps_pair.tile([128, 1024], F32, tag="pair", name=f"st{h}_{comp}_{tk}")
                ksl = slice(tk * 128, (tk + 1) * 128)
                mm(st[:, 0:512], ks_t[:, ksl], qs_t[:, 0:512], True, True)
                mm(st[:, 512:1024], ks_t[:, ksl], qs_t[:, 512:1024], True, True)
                e = pE.tile([128, 1024], BF16, tag="e", name=f"e{h}_{comp}_{tk}")
                nc.scalar.activation(e, st, func=EXP)
                # bf16 pairwise e-sums on the DVE halve the ones-matmuls
                if tk % 2 == 1:
                    es = pEs.tile([128, 1024], BF16, tag="es", name=f"es{h}_{comp}_{tk}")
                    nc.vector.tensor_add(es, e_prev, e)
                if tk == 0 and filler is not None:
                    filler()
                if tk == 0 and state["post"] is not None:
                    flush_post()
                else:
                    flush()
                if tk % 2 == 1:
                    # lag-2: previous pair's sums matmuls go out now
                    if sums_pend is not None:
                        emit_sums(sums_pend)
                    sums_pend = (sums, es, tk == 1, tk == TC - 1)
                if comp == 0:
                    vt = v1[:, tk * 1024 + h * 128 : tk * 1024 + h * 128 + 128]
                else:
                    vt = v2t[:, tk * 128 : (tk + 1) * 128]
                state["pending"] = (p0, p1, e, vt, tk == 0, tk == TC - 1)
                e_prev = e
            state["post"] = (sums_pend, make_boundary(h, comp, sums, pp, recs, psbs))

        def emit_norm(h, recs, psbs):
            osl = slice(h * 1024, (h + 1) * 1024)
            t1 = pEs.tile([128, 1024], BF16, tag="es", name=f"t1_{h}")
            nc.vector.tensor_mul(t1, psbs[0], recs[0])
            nc.vector.tensor_mul(osb[:, osl], psbs[1], recs[1])
            nc.vector.tensor_add(osb[:, osl], osb[:, osl], t1)

        # prologue: head 0 projections
        prefetch_w(0)
        for kk in range(KC):
            tk_ = pB.tile([128, S], MDT, tag="bigB", name=f"xtk{kk}")
            dma_split(tk_, d_xtk[kk], n=2)
            xtk.append(tk_)
        prefetch_w(1)
        for kk in range(KC):
            tq = pA.tile([128, S], MDT, tag="bigA", name=f"xtq{kk}")
            dma_split(tq, d_xtq[kk], n=2)
            xtq.append(tq)
        kp = emit_k_chain(0)
        ks0, kn0 = finish_k(0, kp)
        qp = emit_q_chain(0)
        qs0, qw0 = finish_q(0, qp)
        cur = (qs0, qw0, ks0, kn0, build_v2(0))

        kp_box = {}
        for h in range(H):
            prefetch_w(h + 2)
            recs, psbs = [], []
            emit_comp(h, 0, *cur, recs, psbs)
            # splice the next head's K chain into the comp0->comp1 boundary
            # (PE filler while exp(c0,7) + ln/rec complete)
            filler = None
            if h + 1 < H:
                def filler(hh=h + 1):
                    # K chain as PE filler; its evacuation + kneg go on the
                    # DVE queue ahead of comp1's e-sum adds so the next
                    # head's first score matmul is never gated on them.
                    kp = emit_k_chain(hh)
                    kp_box["kn"] = finish_k(hh, kp)
            emit_comp(h, 1, *cur, recs, psbs, filler=filler)
            if h + 1 < H:
                ksn, knn = kp_box.pop("kn")
                qp = emit_q_chain(h + 1)  # PE filler for the c1-iter7 flush
                qsn, qwn = finish_q(h + 1, qp)  # DVE casts ahead of P evac
                flush_post()  # c1 final AV+sums + boundary (rec_i, P2 evac)
                nxt = (qsn, qwn, ksn, knn, build_v2(h + 1))
            else:
                # last head: prefetch wo during the tail
                wo_t = []
                for hh in range(H):
                    tw = pA.tile([128, 1024], BF16, tag="bigA", name=f"wo{hh}")
                    nc.sync.dma_start(out=tw, in_=d_wo[hh])
                    wo_t.append(tw)
                flush_post()
                nxt = None
            emit_norm(h, recs, psbs)
            cur = nxt

        # ---- output projection (transposed: out[odim, tok]) ----
        for oc in range(KC):
            pair = ps_pair.tile([128, 1024], F32, tag="pair", name=f"ops{oc}")
            osl = slice(oc * 128, (oc + 1) * 128)
            for hh in range(H):
                lhsT = wo_t[hh][:, osl]
                hb = hh * 1024
                mm(pair[:, 0:512], lhsT, osb[:, hb : hb + 512], hh == 0, hh == H - 1)
                mm(
                    pair[:, 512:1024],
                    lhsT,
                    osb[:, hb + 512 : hb + 1024],
                    hh == 0,
                    hh == H - 1,
                )
            oev = pB.tile([128, 1024], BF16, tag="bigB", name=f"oev{oc}")
            if oc % 2 == 0:
                nc.scalar.copy(oev, pair)
            else:
                nc.vector.tensor_copy(oev, pair)
            nc.sync.dma_start(out=d_out[oc][:, 0:512], in_=oev[:, 0:512])
            nc.sync.dma_start(out=d_out[oc][:, 512:1024], in_=oev[:, 512:1024])

    _split_waits(nc)
    return nc


_NC_CACHE = {}


def kernel(
    queries,
    keys,
    values,
    wq_r,
    wq_i,
    wk_r,
    wk_i,
    wv_r,
    wv_i,
    wo_r,
    wo_i,
    _trace=False,
):
    global LAST_EXEC_NS
    _install_axon_profile_shim()
    _install_tile_drain_patch()
    from concourse.bass_utils import run_bass_kernel_spmd

    scale = 1.0 / np.sqrt(DH)
    WQ = _head_tiles(_build_wqk(np.asarray(wq_r), np.asarray(wq_i), scale))
    WK = _head_tiles(_build_wqk(np.asarray(wk_r), np.asarray(wk_i), 1.0))
    import ml_dtypes
    WV = _kchunk_tiles(_build_wqk(np.asarray(wv_r), np.asarray(wv_i), 1.0)).astype(
        ml_dtypes.bfloat16
    )
    WO = _kchunk_tiles(_build_wo(np.asarray(wo_r), np.asarray(wo_i))).astype(
        ml_dtypes.bfloat16
    )
    CST = np.zeros((128, 320), np.float32)
    CST[:, 0:128] = 1.0
    CSTB = np.ones((128, 128), ml_dtypes.bfloat16)

    queries = np.asarray(queries)
    keys = np.asarray(keys)
    values = np.asarray(values)

    in_maps = []
    for b in range(NCORES):
        in_maps.append(
            {
                "xtq": _xt(queries[b]).reshape(KC, 128, S),
                "xtk": _xt(keys[b]).reshape(KC, 128, S),
                "xtv": _xt(values[b]).reshape(KC, 128, S).astype(ml_dtypes.bfloat16),
                "wq": WQ,
                "wk": WK,
                "wv": WV,
                "wo": WO,
                "cst": CST,
                "cstb": CSTB,
            }
        )

    if "nc" not in _NC_CACHE:
        _NC_CACHE["nc"] = _build_nc()
    nc = _NC_CACHE["nc"]

    res = run_bass_kernel_spmd(nc, in_maps, list(range(NCORES)), trace=_trace)
    LAST_EXEC_NS = res.exec_time_ns

    out = np.empty((B, S, D, 2), np.float32)
    for b in range(NCORES):
        # res: [oc, odim, tok] -> [tok, oc*128+odim] -> [S, D, 2]
        r = np.asarray(res.results[b]["out"], np.float32).reshape(1024, S)
        out[b] = r.T.reshape(S, D, 2)
    return out


# revision 18
# speedup vs baseline: 1.2530x; 1.0135x over previous
"""Complex multi-head attention on 8 Trainium2 cores (Bass/Tile).

Sharding: pure data-parallel over batch (B=8 -> 1 batch per core),
weights replicated. No collectives.

v2 redesign (vs baseline at 465us):
  - Every LDWEIGHTS serves 2 matmuls (the two 512-col halves of each
    1024-wide rhs share the stationary operand) so the fp32 weight-load
    (224ns, no FWL for fp32) hides under 2x213ns of streaming.
  - Scores for both halves land in one 2-bank PSUM pair tile and are
    exponentiated with a single [128,1024] activation (amortizes the
    ~300-cycle ACT instruction overhead).
  - Softmax reciprocal moved off the DVE (was 32 x 3.9us of iterative
    divide) to the scalar engine as exp(-ln(sums)); Ln+Exp share one
    ACT table set.
  - P1/P2 evacuated unnormalized; normalization (3 DVE ops/head)
    happens off the critical path during the next head's projections.
  - Lag-1 software pipeline: st pair of iter i+1 is emitted between
    exp(i) and the sums/AV matmuls of iter i, so the PE never waits on
    the scalar engine; next head's K/Q projection chains are spliced
    into the head boundary to cover the last exp + rec latency.
  - O-projection computed transposed (out = [odim, tok]) with
    wo-stationary h-chains so its LDW is also paired; host undoes the
    transpose.
PSUM: st/proj/V/O pair pool [128,1024]x2 (4 banks) + sums pair
[128,1024]x1 (2) + P accumulators [128,512]x2 (2) = 8 banks exactly.
"""

import sys
import types
import numpy as np

B, S, D, H = 8, 1024, 512, 8
DH = D // H
KC = 8  # k-chunks of 128 over (c,d) = 1024
TC = 8  # token chunks of 128
NCORES = 8

LAST_EXEC_NS = None


# ---------------------------------------------------------------- shims
def _install_axon_profile_shim():
    if "antenv.axon_hooks" in sys.modules:
        return
    try:
        import antenv  # noqa: F401

        mod = types.ModuleType("antenv.axon_hooks")
        state = {"hook": None}
        mod.set_axon_ntff_profile_hook = lambda h: state.__setitem__("hook", h)
        mod.get_axon_ntff_profile_hook = lambda: state["hook"]
        sys.modules["antenv.axon_hooks"] = mod
        from trn_agent_boot.trn_boot import _ntff_profile_via_ctypes

        hook = _ntff_profile_via_ctypes("/opt/axon/libaxon_pjrt.so")
        if hook is not None:
            mod.set_axon_ntff_profile_hook(hook)
    except Exception:
        pass


def _install_tile_drain_patch():
    """This walrus build allows ONE sync wait per instruction; split the
    TileContext exit drain's waits across preceding sync NOPs."""
    import concourse.mybir as mybir
    import concourse.tile as tile
    from concourse.vector_clock import ScopedClock

    if getattr(tile.TileContext, "_drain_patched", False):
        return

    def _patched(self, tick_clock, wait_clock):
        probe = mybir.InstNoOp(name="I-drain-probe")
        probe.engine = mybir.EngineType.SP
        wait_clock.add_sem_waits(probe, ScopedClock({None: tick_clock.global_clock}))
        waits = list(probe.sync_info.on_wait or []) if probe.sync_info else []
        for w in waits:
            nop = self.nc.sync.nop()
            nop.ins.sync_info = mybir.SyncInfo(on_wait=[w], on_update=[])
        self.nc.sync.drain()
        self.nc.all_engine_barrier()
        assert self.sems is not None
        popped = self.nc._tile_sem_poison_stack.pop()
        assert popped is self._sem_poison
        self.nc.clear_and_free_semaphores(list(self.sems.allocated().values()))
        self.nc.all_engine_barrier()

    tile.TileContext._drain_and_barrier = _patched
    tile.TileContext._drain_patched = True


def _split_waits(nc, max_waits=1):
    """Hoist extra sync waits onto preceding same-engine NOPs (walrus here
    rejects >1 sync wait per instruction)."""
    import concourse.mybir as mybir

    def process(blk):
        lst = blk.instructions
        i = 0
        while i < len(lst):
            inst = lst[i]
            if hasattr(inst, "blocks"):
                for b in inst.blocks or []:
                    process(b)
            si = inst.sync_info
            if si is not None and si.on_wait and len(si.on_wait) > max_waits:
                waits = list(si.on_wait)
                keep, extra = waits[-max_waits:], waits[:-max_waits]
                inst.sync_info = mybir.SyncInfo(
                    on_wait=keep, on_update=list(si.on_update or [])
                )
                for j, w in enumerate(extra):
                    nop = mybir.InstNoOp(name=f"{inst.name}-ws{j}")
                    nop.engine = inst.engine
                    nop.sync_info = mybir.SyncInfo(on_wait=[w], on_update=[])
                    lst.insert(i, nop)
                    i += 1
            i += 1

    for f in nc.m.functions:
        for blk in f.blocks:
            process(blk)


# ------------------------------------------------------------ host prep
def _build_wqk(wr, wi, scale):
    """[1024 k=(c,d), 1024 m=(h, c', dh)] for Q/K projections."""
    W = np.empty((2 * D, 2 * D), np.float32)
    for h in range(H):
        o = slice(h * DH, (h + 1) * DH)
        c0 = h * 2 * DH
        W[0:D, c0 : c0 + DH] = wr[o].T * scale
        W[D:, c0 : c0 + DH] = -wi[o].T * scale
        W[0:D, c0 + DH : c0 + 2 * DH] = wi[o].T * scale
        W[D:, c0 + DH : c0 + 2 * DH] = wr[o].T * scale
    return W


def _head_tiles(W):
    """[1024,1024] -> [H, 128, 1024]: per-head column block, k-chunk cols."""
    out = np.empty((H, 128, 1024), np.float32)
    for h in range(H):
        blk = W[:, h * 128 : (h + 1) * 128]  # [1024, 128]
        for kk in range(KC):
            out[h, :, kk * 128 : (kk + 1) * 128] = blk[kk * 128 : (kk + 1) * 128]
    return out


def _kchunk_tiles(W):
    """[1024,1024] -> [KC, 128, 1024]: row chunks."""
    return np.ascontiguousarray(W.reshape(KC, 128, 1024))


def _build_wo(wo_r, wo_i):
    """rows (h, c', dh), cols (o, c) interleaved to match [S, D, 2]."""
    W = np.empty((2 * D, 2 * D), np.float32)
    for h in range(H):
        d = slice(h * DH, (h + 1) * DH)
        r0 = h * 2 * DH
        W[r0 : r0 + DH, 0::2] = wo_r[:, d].T
        W[r0 : r0 + DH, 1::2] = wo_i[:, d].T
        W[r0 + DH : r0 + 2 * DH, 0::2] = -wo_i[:, d].T
        W[r0 + DH : r0 + 2 * DH, 1::2] = wo_r[:, d].T
    return W


def _xt(x):  # [S, D, 2] -> [2D, S] feature-major
    out = np.empty((2 * D, S), np.float32)
    out[0:D] = x[:, :, 0].T
    out[D:] = x[:, :, 1].T
    return out


# ------------------------------------------------------------ bass build
def _build_nc():
    import concourse.bass as bass
    import concourse.mybir as mybir
    import concourse.tile as tile
    from contextlib import ExitStack

    MDT = mybir.dt.float32r
    F32 = mybir.dt.float32
    BF16 = mybir.dt.bfloat16
    EXP = mybir.ActivationFunctionType.Exp
    LN = mybir.ActivationFunctionType.Ln

    nc = bass.Bass()
    d_xtq = nc.dram_tensor("xtq", [KC, 128, S], BF16, kind="ExternalInput")
    d_xtk = nc.dram_tensor("xtk", [KC, 128, S], BF16, kind="ExternalInput")
    d_xtv = nc.dram_tensor("xtv", [KC, 128, S], BF16, kind="ExternalInput")
    d_wq = nc.dram_tensor("wq", [H, 128, 1024], BF16, kind="ExternalInput")
    d_wk = nc.dram_tensor("wk", [H, 128, 1024], BF16, kind="ExternalInput")
    d_wv = nc.dram_tensor("wv", [KC, 128, 1024], BF16, kind="ExternalInput")
    d_wo = nc.dram_tensor("wo", [H, 128, 1024], BF16, kind="ExternalInput")
    d_cst = nc.dram_tensor("cst", [128, 320], MDT, kind="ExternalInput")
    d_cstb = nc.dram_tensor("cstb", [128, 128], BF16, kind="ExternalInput")
    # transposed output: [oc, odim, tok]
    d_out = nc.dram_tensor("out", [KC, 128, S], BF16, kind="ExternalOutput")

    with tile.TileContext(nc) as tc, ExitStack() as ctx:
        ctx.enter_context(
            nc.allow_low_precision(reason="float32r tiles are bit-identical fp32")
        )
        pA = ctx.enter_context(tc.tile_pool(name="bigA", bufs=8))  # xtv->xtq->wo
        pB = ctx.enter_context(tc.tile_pool(name="bigB", bufs=8))  # wv->xtk->oev
        pV1 = ctx.enter_context(tc.tile_pool(name="v1", bufs=1))
        pOsb = ctx.enter_context(tc.tile_pool(name="osb", bufs=1))
        pQs = ctx.enter_context(tc.tile_pool(name="qs", bufs=2))
        pKs = ctx.enter_context(tc.tile_pool(name="ks", bufs=2))
        pQw = ctx.enter_context(tc.tile_pool(name="qw", bufs=1))
        pKn = ctx.enter_context(tc.tile_pool(name="kn", bufs=1))
        pV2 = ctx.enter_context(tc.tile_pool(name="v2", bufs=1))
        pE = ctx.enter_context(tc.tile_pool(name="e", bufs=6))
        pEs = ctx.enter_context(tc.tile_pool(name="es", bufs=6))
        pPsb = ctx.enter_context(tc.tile_pool(name="psb", bufs=3))
        pRec = ctx.enter_context(tc.tile_pool(name="rec", bufs=4))
        pWqk = ctx.enter_context(tc.tile_pool(name="wqk", bufs=4))
        pC = ctx.enter_context(tc.tile_pool(name="const", bufs=1))

        ps_pair = ctx.enter_context(tc.tile_pool(name="ps_pair", bufs=2, space="PSUM"))
        ps_sums = ctx.enter_context(tc.tile_pool(name="ps_sums", bufs=1, space="PSUM"))
        ps_p = ctx.enter_context(tc.tile_pool(name="ps_p", bufs=1, space="PSUM"))

        cst = pC.tile([128, 320], MDT, tag="cst")
        nc.sync.dma_start(out=cst, in_=d_cst[:, :])
        onesb = pC.tile([128, 128], BF16, tag="cstb")
        nc.sync.dma_start(out=onesb, in_=d_cstb[:, :])

        def mm(out, lhsT, rhs, start, stop):
            nc.tensor.matmul(out, lhsT=lhsT, rhs=rhs, start=start, stop=stop)

        # ---- phase V: V projection (all heads) ----
        def dma_split(out, in_, n=4):
            w = out.shape[-1] // n
            for i in range(n):
                nc.sync.dma_start(
                    out=out[:, i * w : (i + 1) * w], in_=in_[:, i * w : (i + 1) * w]
                )

        xtv, wv = [], []
        for kk in range(KC):
            tv = pA.tile([128, S], BF16, tag="bigA", name=f"xtv{kk}")
            nc.sync.dma_start(out=tv, in_=d_xtv[kk])
            xtv.append(tv)
            tw = pB.tile([128, 1024], BF16, tag="bigB", name=f"wv{kk}")
            nc.sync.dma_start(out=tw, in_=d_wv[kk])
            wv.append(tw)

        v1 = pV1.tile([128, TC * 1024], BF16, tag="v1")  # [p, (t_, h, dh2)]
        for t_ in range(TC):
            pair = ps_pair.tile([128, 1024], F32, tag="pair", name=f"vps{t_}")
            tsl = slice(t_ * 128, (t_ + 1) * 128)
            for kk in range(KC):
                lhsT = xtv[kk][:, tsl]
                mm(pair[:, 0:512], lhsT, wv[kk][:, 0:512], kk == 0, kk == KC - 1)
                mm(pair[:, 512:1024], lhsT, wv[kk][:, 512:1024], kk == 0, kk == KC - 1)
            nc.vector.tensor_copy(v1[:, t_ * 1024 : (t_ + 1) * 1024], pair)

        # ---- load XT_q / XT_k (reuse pA / pB slots) ----
        xtq, xtk = [], []

        osb = pOsb.tile([128, H * 1024], BF16, tag="osb")  # [p, (h, tok)]

        # ---------------- per-head helpers ----------------
        wq_t, wk_t = {}, {}

        def prefetch_w(h):
            if h >= H or h in wq_t:
                return
            tk2 = pWqk.tile([128, 1024], BF16, tag="wqk", name=f"wk{h}")
            dma_split(tk2, d_wk[h], n=2)
            wk_t[h] = tk2
            tq = pWqk.tile([128, 1024], BF16, tag="wqk", name=f"wq{h}")
            dma_split(tq, d_wq[h], n=2)
            wq_t[h] = tq

        def proj_chain(w_tile, xt_tiles, name):
            pair = ps_pair.tile([128, 1024], F32, tag="pair", name=name)
            for kk in range(KC):
                lhsT = w_tile[:, kk * 128 : (kk + 1) * 128]
                mm(pair[:, 0:512], lhsT, xt_tiles[kk][:, 0:512], kk == 0, kk == KC - 1)
                mm(
                    pair[:, 512:1024],
                    lhsT,
                    xt_tiles[kk][:, 512:1024],
                    kk == 0,
                    kk == KC - 1,
                )
            return pair

        def emit_k_chain(h):
            return proj_chain(wk_t.pop(h), xtk, f"kproj{h}")

        def finish_k(h, pair):
            ks = pKs.tile([128, S], BF16, tag="ks", name=f"kstack{h}")
            nc.vector.tensor_copy(ks, pair)
            kn = pKn.tile([128, S], BF16, tag="kn", name=f"kneg{h}")
            nc.vector.tensor_copy(kn[0:64, :], ks[0:64, :])
            nc.vector.tensor_scalar_mul(kn[64:128, :], ks[64:128, :], -1.0)
            return ks, kn

        def emit_q_chain(h):
            return proj_chain(wq_t.pop(h), xtq, f"qproj{h}")

        def finish_q(h, pair):
            qs = pQs.tile([128, S], BF16, tag="qs", name=f"qstack{h}")
            nc.vector.tensor_copy(qs[:, 0:512], pair[:, 0:512])
            nc.vector.tensor_copy(qs[:, 512:1024], pair[:, 512:1024])
            qw = pQw.tile([128, S], BF16, tag="qw", name=f"qswap{h}")
            nc.sync.dma_start(out=qw[0:64, :], in_=qs[64:128, :])
            nc.sync.dma_start(out=qw[64:128, :], in_=qs[0:64, :])
            return qs, qw

        def build_v2(h):
            v2t = pV2.tile([128, 1024], BF16, tag="v2", name=f"v2h{h}")
            v1v = v1.rearrange("p (t h d) -> p t h d", t=TC, h=H, d=128)
            v2v = v2t.rearrange("p (t d) -> p t d", t=TC, d=128)
            nc.vector.tensor_scalar_mul(
                v2v[:, :, 0:64], v1v[:, :, h, 64:128], -1.0
            )
            nc.vector.tensor_copy(v2v[:, :, 64:128], v1v[:, :, h, 0:64])
            return v2t

        # ---------------- the pipelined head loop ----------------
        # pending = (sums, p0, p1, e, vt, first, last, boundary_cb)
        state = {"pending": None, "post": None}

        def flush_post():
            flush()
            sp, boundary = state["post"]
            emit_sums(sp)
            boundary()
            state["post"] = None

        def emit_sums(sp):
            sums, es, first, last = sp
            mm(sums[:, 0:512], onesb, es[:, 0:512], first, last)
            mm(sums[:, 512:1024], onesb, es[:, 512:1024], first, last)

        def flush():
            p = state["pending"]
            if p is None:
                return
            p0, p1, e, vt, first, last = p
            mm(p0, vt, e[:, 0:512], first, last)
            mm(p1, vt, e[:, 512:1024], first, last)
            state["pending"] = None

        def make_boundary(h, comp, sums, pp, recs, psbs):
            def boundary():
                lnt = pRec.tile([128, 1024], MDT, tag="rec", name=f"lnt{h}_{comp}")
                nc.scalar.activation(lnt, sums, func=LN)
                rec = pRec.tile([128, 1024], MDT, tag="rec", name=f"rec{h}_{comp}")
                nc.scalar.activation(rec, lnt, func=EXP, scale=-1.0)
                recs.append(rec)
                psb = pPsb.tile([128, 1024], MDT, tag="psb", name=f"psb{h}_{comp}")
                nc.vector.tensor_copy(psb, pp)
                psbs.append(psb)

            return boundary

        def emit_comp(h, comp, qs, qw, ks, kn, v2t, recs, psbs, filler=None):
            ks_t = kn if comp == 0 else ks
            qs_t = qs if comp == 0 else qw
            sums = ps_sums.tile([128, 1024], F32, tag="sums", name=f"sums{h}_{comp}")
            pp = ps_p.tile([128, 1024], F32, tag="p", name=f"pp{h}_{comp}")
            p0 = pp[:, 0:512]
            p1 = pp[:, 512:1024]
            e_prev = None
            sums_pend = None
            for tk in range(TC):
                st = ps_pair.tile([128, 1024], F32, tag="pair", name=f"st{h}_{comp}_{tk}")
                ksl = slice(tk * 128, (tk + 1) * 128)
                mm(st[:, 0:512], ks_t[:, ksl], qs_t[:, 0:512], True, True)
                mm(st[:, 512:1024], ks_t[:, ksl], qs_t[:, 512:1024], True, True)
                e = pE.tile([128, 1024], BF16, tag="e", name=f"e{h}_{comp}_{tk}")
                nc.scalar.activation(e, st, func=EXP)
                # bf16 pairwise e-sums on the DVE halve the ones-matmuls
                if tk % 2 == 1:
                    es = pEs.tile([128, 1024], BF16, tag="es", name=f"es{h}_{comp}_{tk}")
                    nc.vector.tensor_add(es, e_prev, e)
                if tk == 0 and filler is not None:
                    filler()
                if tk == 0 and state["post"] is not None:
                    flush_post()
                else:
                    flush()
                if tk % 2 == 1:
                    # lag-2: previous pair's sums matmuls go out now
                    if sums_pend is not None:
                        emit_sums(sums_pend)
                    sums_pend = (sums, es, tk == 1, tk == TC - 1)
                if comp == 0:
                    vt = v1[:, tk * 1024 + h * 128 : tk * 1024 + h * 128 + 128]
                else:
                    vt = v2t[:, tk * 128 : (tk + 1) * 128]
                state["pending"] = (p0, p1, e, vt, tk == 0, tk == TC - 1)
                e_prev = e
            state["post"] = (sums_pend, make_boundary(h, comp, sums, pp, recs, psbs))

        def emit_norm(h, recs, psbs):
            osl = slice(h * 1024, (h + 1) * 1024)
            t1 = pEs.tile([128, 1024], BF16, tag="es", name=f"t1_{h}")
            nc.vector.tensor_mul(t1, psbs[0], recs[0])
            nc.vector.tensor_mul(osb[:, osl], psbs[1], recs[1])
            nc.vector.tensor_add(osb[:, osl], osb[:, osl], t1)

        # prologue: head 0 projections
        prefetch_w(0)
        for kk in range(KC):
            tk_ = pB.tile([128, S], BF16, tag="bigB", name=f"xtk{kk}")
            dma_split(tk_, d_xtk[kk], n=2)
            xtk.append(tk_)
        prefetch_w(1)
        for kk in range(KC):
            tq = pA.tile([128, S], BF16, tag="bigA", name=f"xtq{kk}")
            dma_split(tq, d_xtq[kk], n=2)
            xtq.append(tq)
        kp = emit_k_chain(0)
        ks0, kn0 = finish_k(0, kp)
        qp = emit_q_chain(0)
        qs0, qw0 = finish_q(0, qp)
        cur = (qs0, qw0, ks0, kn0, build_v2(0))

        kp_box = {}
        for h in range(H):
            prefetch_w(h + 2)
            recs, psbs = [], []
            emit_comp(h, 0, *cur, recs, psbs)
            # splice the next head's K chain into the comp0->comp1 boundary
            # (PE filler while exp(c0,7) + ln/rec complete)
            filler = None
            if h + 1 < H:
                def filler(hh=h + 1):
                    # K chain as PE filler; its evacuation + kneg go on the
                    # DVE queue ahead of comp1's e-sum adds so the next
                    # head's first score matmul is never gated on them.
                    kp = emit_k_chain(hh)
                    kp_box["kn"] = finish_k(hh, kp)
            emit_comp(h, 1, *cur, recs, psbs, filler=filler)
            if h + 1 < H:
                ksn, knn = kp_box.pop("kn")
                qp = emit_q_chain(h + 1)  # PE filler for the c1-iter7 flush
                qsn, qwn = finish_q(h + 1, qp)  # DVE casts ahead of P evac
                flush_post()  # c1 final AV+sums + boundary (rec_i, P2 evac)
                nxt = (qsn, qwn, ksn, knn, build_v2(h + 1))
            else:
                # last head: prefetch wo during the tail
                wo_t = []
                for hh in range(H):
                    tw = pA.tile([128, 1024], BF16, tag="bigA", name=f"wo{hh}")
                    nc.sync.dma_start(out=tw, in_=d_wo[hh])
                    wo_t.append(tw)
                flush_post()
                nxt = None
            emit_norm(h, recs, psbs)
            cur = nxt

        # ---- output projection (transposed: out[odim, tok]) ----
        for oc in range(KC):
            pair = ps_pair.tile([128, 1024], F32, tag="pair", name=f"ops{oc}")
            osl = slice(oc * 128, (oc + 1) * 128)
            for hh in range(H):
                lhsT = wo_t[hh][:, osl]
                hb = hh * 1024
                mm(pair[:, 0:512], lhsT, osb[:, hb : hb + 512], hh == 0, hh == H - 1)
                mm(
                    pair[:, 512:1024],
                    lhsT,
                    osb[:, hb + 512 : hb + 1024],
                    hh == 0,
                    hh == H - 1,
                )
            oev = pB.tile([128, 1024], BF16, tag="bigB", name=f"oev{oc}")
            if oc % 2 == 0:
                nc.scalar.copy(oev, pair)
            else:
                nc.vector.tensor_copy(oev, pair)
            nc.sync.dma_start(out=d_out[oc][:, 0:512], in_=oev[:, 0:512])
            nc.sync.dma_start(out=d_out[oc][:, 512:1024], in_=oev[:, 512:1024])

    _split_waits(nc)
    return nc


_NC_CACHE = {}


def kernel(
    queries,
    keys,
    values,
    wq_r,
    wq_i,
    wk_r,
    wk_i,
    wv_r,
    wv_i,
    wo_r,
    wo_i,
    _trace=False,
):
    global LAST_EXEC_NS
    _install_axon_profile_shim()
    _install_tile_drain_patch()
    from concourse.bass_utils import run_bass_kernel_spmd

    scale = 1.0 / np.sqrt(DH)
    import ml_dtypes
    WQ = _head_tiles(_build_wqk(np.asarray(wq_r), np.asarray(wq_i), scale)).astype(
        ml_dtypes.bfloat16
    )
    WK = _head_tiles(_build_wqk(np.asarray(wk_r), np.asarray(wk_i), 1.0)).astype(
        ml_dtypes.bfloat16
    )
    WV = _kchunk_tiles(_build_wqk(np.asarray(wv_r), np.asarray(wv_i), 1.0)).astype(
        ml_dtypes.bfloat16
    )
    WO = _kchunk_tiles(_build_wo(np.asarray(wo_r), np.asarray(wo_i))).astype(
        ml_dtypes.bfloat16
    )
    CST = np.zeros((128, 320), np.float32)
    CST[:, 0:128] = 1.0
    CSTB = np.ones((128, 128), ml_dtypes.bfloat16)

    queries = np.asarray(queries)
    keys = np.asarray(keys)
    values = np.asarray(values)

    in_maps = []
    for b in range(NCORES):
        in_maps.append(
            {
                "xtq": _xt(queries[b]).reshape(KC, 128, S).astype(ml_dtypes.bfloat16),
                "xtk": _xt(keys[b]).reshape(KC, 128, S).astype(ml_dtypes.bfloat16),
                "xtv": _xt(values[b]).reshape(KC, 128, S).astype(ml_dtypes.bfloat16),
                "wq": WQ,
                "wk": WK,
                "wv": WV,
                "wo": WO,
                "cst": CST,
                "cstb": CSTB,
            }
        )

    if "nc" not in _NC_CACHE:
        _NC_CACHE["nc"] = _build_nc()
    nc = _NC_CACHE["nc"]

    res = run_bass_kernel_spmd(nc, in_maps, list(range(NCORES)), trace=_trace)
    LAST_EXEC_NS = res.exec_time_ns

    out = np.empty((B, S, D, 2), np.float32)
    for b in range(NCORES):
        # res: [oc, odim, tok] -> [tok, oc*128+odim] -> [S, D, 2]
        r = np.asarray(res.results[b]["out"], np.float32).reshape(1024, S)
        out[b] = r.T.reshape(S, D, 2)
    return out
